# revision 1
# baseline (speedup 1.0000x reference)
"""Trainium2 Bass kernel for a single-head AttentionBlock with residual.

Reference computation (per batch b):
    q = x @ Wq^T + bq ; k = x @ Wk^T + bk ; v = x @ Wv^T + bv
    s = (q @ k^T) / sqrt(D)         [S, S]
    s = where(mask[b] == 0 (keys), -1e10, s)
    a = softmax(s, axis=-1)
    out = x + (a @ v) @ Wo^T + bo

Sharding: 8 cores = 4 batches x 2 query-halves. With DEDUP=True (default)
each core projects K/V only for its own key half (== its query rows) and the
two cores of a batch exchange halves via a 2-member AllGather through a DRAM
bounce, so no projection work is duplicated (17.2 GFLOP/core, the fair
share). A tiny dummy AllGather issued at kernel start absorbs the ~70us
boot-once dispatch latency of the collectives core so the real exchanges
start as soon as their inputs are staged.

Device-side layout (per core, P = 128 partitions):
    xt   [D, SKV]  x^T for K/V projections (moving / stationary operand)
    xqt  [D, SQ]   x^T restricted to this core's query rows
    QT   [e, q] = wqT.T-matmul   (scores lhsT)
    KT   [e, k]                  (scores lhsT per key tile)
    V    [k, e(+1 ones col)]     (O-pass stationary; ones col -> row sums)
    scoresT[k, q] -> exp(.+mask_bias_k) -> expT  (mask bias is per-partition)
    O^T_unnorm [e, q] + rsum [1, q]  accumulated in PSUM over key tiles
    out[q, f] = hs[q, f] + (O^T.T @ woT) * (1/rsum)[q]

Softmax max-subtraction is skipped: scores are ~N(0,1) here (exp < ~200),
fp32 exp is safe. Masked keys get bias -30000 -> exp underflows to exactly 0.

bq/bk are assumed zero (spec fill=zeros); nonzero triggers an exact numpy
fallback. bv/bo are folded into the residual on the host (exact).
"""

import functools
from contextlib import ExitStack

import numpy as np

import concourse.bass as bass
import concourse.tile as tile
from concourse import bacc, mybir
from concourse.bass_utils import run_bass_kernel_spmd

P = 128
NEG_BIAS = -30000.0
N_CORES = 8


def _chunks(total, size):
    return [(o, min(size, total - o)) for o in range(0, total, size)]


def build_program(D=1024, SQ=1024, SKV=2048, mmdt=mybir.dt.float16,
                  dedup=False, n_cores=8):
    """Build + compile the single-core Bass program (same program on all cores).

    dedup=True: each core projects K/V only for its local key half (== its
    query rows) and the halves are exchanged with the paired core via a
    2-member AllGather (DRAM bounce). Saves 1/5 of the matmul work.
    """
    f32 = mybir.dt.float32
    DT = D // P    # d contraction tiles
    ET = D // P    # e tiles
    KT = SKV // P  # key tiles
    QT = SQ // P   # query row tiles

    nc = bacc.Bacc("TRN2", target_bir_lowering=False, debug=False,
                   num_devices=n_cores)

    if not dedup:
        xt_d = nc.dram_tensor("xt", [D, SKV], mmdt, kind="ExternalInput")
    xqt_d = nc.dram_tensor("xqt", [D, SQ], mmdt, kind="ExternalInput")
    hs_d = nc.dram_tensor("hs", [SQ, D], f32, kind="ExternalInput")
    wq_d = nc.dram_tensor("wq", [D, D], mmdt, kind="ExternalInput")
    wk_d = nc.dram_tensor("wk", [D, D], mmdt, kind="ExternalInput")
    wv_d = nc.dram_tensor("wv", [D, D], mmdt, kind="ExternalInput")
    wo_d = nc.dram_tensor("wo", [D, D], mmdt, kind="ExternalInput")
    mb_d = nc.dram_tensor("mb", [P, KT], f32, kind="ExternalInput")
    out_d = nc.dram_tensor("out", [SQ, D], f32, kind="ExternalOutput")

    Exp = mybir.ActivationFunctionType.Exp
    mult = mybir.AluOpType.mult
    add = mybir.AluOpType.add

    with tile.TileContext(nc) as tc, ExitStack() as ctx:
        # big tensors that alternate in time share 4MB-slot tags
        bigA = ctx.enter_context(tc.tile_pool(name="bigA", bufs=2))
        qk_pool = ctx.enter_context(tc.tile_pool(name="qk", bufs=1))
        v_pool = ctx.enter_context(tc.tile_pool(name="vp", bufs=1))
        wpool = ctx.enter_context(tc.tile_pool(name="w", bufs=2))
        con = ctx.enter_context(tc.tile_pool(name="const", bufs=1))
        outp = ctx.enter_context(tc.tile_pool(name="outs", bufs=2))

        pp = ctx.enter_context(tc.tile_pool(name="pp", bufs=5, space="PSUM"))
        rsp = ctx.enter_context(tc.tile_pool(name="rsp", bufs=1, space="PSUM"))

        # ---- PE warmup during the initial DMA wait (HAM ramp) ----
        ones1h = con.tile([1, 1], mmdt)
        nc.gpsimd.memset(ones1h[:], 1.0)
        warm_in = con.tile([1, 256], mmdt)
        nc.gpsimd.memset(warm_in[:], 0.0)
        warm_ps = pp.tile([P, 512], f32, tag="pp")
        N_WARM = 16
        for i in range(N_WARM):
            nc.tensor.matmul(warm_ps[:1, :256], ones1h[:], warm_in[:],
                             start=(i == 0), stop=(i == N_WARM - 1))
        warm_out = con.tile([1, 256], f32)
        nc.vector.tensor_copy(warm_out[:], warm_ps[:1, :256])

        mb = con.tile([P, KT], f32)
        nc.gpsimd.dma_start(mb[:], mb_d.ap())
        ones1 = con.tile([1, 1], f32)
        nc.gpsimd.memset(ones1[:], 1.0)

        # ---- first-needed loads first; split across the three DMA-capable
        # queues (per-queue streaming tops out well below HBM bandwidth) ----
        _engs = [nc.gpsimd, nc.sync, nc.scalar]

        def load_w(dram, eng=None, split=1):
            w = wpool.tile([P, DT, D], mmdt, tag="w")
            wv_ = dram.ap().rearrange("(t p) e -> p t e", p=P)
            split = min(split, DT)
            step = DT // split
            for i in range(split):
                e = _engs[i % 3] if eng is None else eng
                sl = slice(i * step, (i + 1) * step)
                e.dma_start(w[:, sl, :], wv_[:, sl, :])
            return w

        xqt = bigA.tile([P, DT, SQ], mmdt, tag="bigA")
        xqt_v = xqt_d.ap().rearrange("(t p) q -> p t q", p=P)

        def load_xqt(off):
            split = min(4, DT)
            step = DT // split
            for i in range(split):
                sl = slice(i * step, (i + 1) * step)
                _engs[(i + off) % 3].dma_start(xqt[:, sl, :], xqt_v[:, sl, :])

        def proj(w, x, xo, xn, et, psn=512):
            """psum <- w[:, :, et].T @ x[:, :, xo:xo+xn] accumulated over DT."""
            ps = pp.tile([P, psn], f32, tag="pp")
            for dt_ in range(DT):
                nc.tensor.matmul(
                    ps[:, :xn], w[:, dt_, et * P:(et + 1) * P],
                    x[:, dt_, xo:xo + xn],
                    start=(dt_ == 0), stop=(dt_ == DT - 1),
                )
            return ps

        kt_sb = qk_pool.tile([P, ET, SKV], mmdt, tag="kt")
        v = v_pool.tile([P, KT, D + 1], mmdt, tag="v")
        nc.gpsimd.memset(v[:, :, D:D + 1], 1.0)  # ones col -> row-sum rows
        qt = qk_pool.tile([P, ET, SQ], mmdt, tag="qt")

        if not dedup:
            wq = load_w(wq_d, split=4)
            load_xqt(1)
            xt = bigA.tile([P, DT, SKV], mmdt, tag="bigA")
            xt_v = xt_d.ap().rearrange("(t p) k -> p t k", p=P)
            xsplit = min(4, DT)
            for i in range(xsplit):
                step = DT // xsplit
                sl = slice(i * step, (i + 1) * step)
                _engs[(i + 2) % 3].dma_start(xt[:, sl, :], xt_v[:, sl, :])
            wk = load_w(wk_d, nc.sync)

            # Q projection: QT[e, q] = wqT.T @ xqt
            for et in range(ET):
                for qo, qn in _chunks(SQ, 512):
                    ps = proj(wq, xqt, qo, qn, et)
                    nc.vector.tensor_copy(qt[:, et, qo:qo + qn], ps[:, :qn])

            # K projection: KT[e, k] = wkT.T @ xt
            for et in range(ET):
                for ko, kn in _chunks(SKV, 512):
                    ps = proj(wk, xt, ko, kn, et)
                    nc.vector.tensor_copy(kt_sb[:, et, ko:ko + kn], ps[:, :kn])

            # V projection (natural [k, e]): V = xt.T @ wvT
            wv = load_w(wv_d, nc.gpsimd)
            for vt in range(KT):
                for eo, en in _chunks(D, 512):
                    ps = pp.tile([P, 512], f32, tag="pp")
                    for dt_ in range(DT):
                        nc.tensor.matmul(
                            ps[:, :en], xt[:, dt_, vt * P:(vt + 1) * P],
                            wv[:, dt_, eo:eo + en],
                            start=(dt_ == 0), stop=(dt_ == DT - 1),
                        )
                    nc.vector.tensor_copy(v[:, vt, eo:eo + en], ps[:, :en])
        else:
            # ---- dedup: project local key half only, AllGather with pair ----
            pairs = [[2 * b, 2 * b + 1] for b in range(n_cores // 2)]
            dram = ctx.enter_context(tc.tile_pool(name="dram", bufs=1, space="DRAM"))
            stg = ctx.enter_context(tc.tile_pool(name="stg", bufs=6))

            # CC-core warmup: the first collective of a kernel dispatches only
            # ~70us in (boot-once cost on the collectives core). Issue a tiny
            # dummy AllGather immediately so the real K/V exchanges dispatch
            # as soon as their inputs are staged.
            ccw_in_d = dram.tile([P, 16], f32, tag="ccwi", name="ccw_in")
            ccw_out_d = dram.tile([2 * P, 16], f32, tag="ccwo", name="ccw_out")
            ccw_sb = con.tile([P, 16], f32)
            nc.gpsimd.memset(ccw_sb[:], 0.0)
            nc.sync.dma_start(ccw_in_d[:], ccw_sb[:])
            nc.gpsimd.collective_compute(
                "AllGather", mybir.AluOpType.bypass, replica_groups=pairs,
                ins=[ccw_in_d[:].opt()], outs=[ccw_out_d[:].opt()],
            )
            SQH = SQ // 2  # K exchange pipelined in two key-column halves
            kt_loc_d = dram.tile([D, SQ], mmdt, tag="ktl", name="kt_loc")
            kt_g_d = dram.tile([2 * D, SQ], mmdt, tag="ktg", name="kt_g")
            v_loc_d = dram.tile([SQ, D], mmdt, tag="vl")
            v_g_d = dram.tile([2 * SQ, D], mmdt, tag="vg")

            wk = load_w(wk_d, split=4)
            load_xqt(1)
            wv = load_w(wv_d, nc.gpsimd)
            wq = load_w(wq_d, nc.gpsimd)

            # K_loc[e, k_loc] = wkT.T @ xqt -> SBUF staging -> DRAM bounce
            ktl_v = kt_loc_d[:].rearrange("(t p) k -> p t k", p=P)
            si = 0
            for et in range(ET):
                for ko, kn in _chunks(SQ, 512):
                    ps = proj(wk, xqt, ko, kn, et)
                    st = stg.tile([P, 512], mmdt, tag="stage")
                    nc.vector.tensor_copy(st[:, :kn], ps[:, :kn])
                    (nc.sync if si % 2 else nc.scalar).dma_start(
                        ktl_v[:, et, ko:ko + kn], st[:, :kn])
                    si += 1
            nc.gpsimd.collective_compute(
                "AllGather", mybir.AluOpType.bypass, replica_groups=pairs,
                ins=[kt_loc_d[:].opt()], outs=[kt_g_d[:].opt()],
            )
            # (the gather-in DMAs are emitted after the V_loc stage-outs:
            # queue order is emission order, and a 2MB gather parked ahead of
            # the staging DMAs stalls eviction slot recycling -> DVE -> PE)

            # V_loc[k_loc, e] = xqt.T @ wvT -> SBUF staging -> DRAM bounce
            vl_v = v_loc_d[:].rearrange("(t p) e -> p t e", p=P)
            for vt in range(SQ // P):
                for eo, en in _chunks(D, 512):
                    ps = pp.tile([P, 512], f32, tag="pp")
                    for dt_ in range(DT):
                        nc.tensor.matmul(
                            ps[:, :en], xqt[:, dt_, vt * P:(vt + 1) * P],
                            wv[:, dt_, eo:eo + en],
                            start=(dt_ == 0), stop=(dt_ == DT - 1),
                        )
                    st = stg.tile([P, 512], mmdt, tag="stage")
                    nc.vector.tensor_copy(st[:, :en], ps[:, :en])
                    (nc.sync if si % 2 else nc.scalar).dma_start(
                        vl_v[:, vt, eo:eo + en], st[:, :en])
                    si += 1
            nc.gpsimd.collective_compute(
                "AllGather", mybir.AluOpType.bypass, replica_groups=pairs,
                ins=[v_loc_d[:].opt()], outs=[v_g_d[:].opt()],
            )
            # K gather-ins: gpsimd is drained by now (wq landed) and the CC
            # warmup means CC_K completes before this point in queue time.
            # 4-way split (pair member x e-tile half) across all three queues
            # so the full 4MB lands before the scores phase consumes it.
            ETH = max(ET // 2, 1)
            ktg_engs = {(0, 0): nc.gpsimd, (0, 1): nc.scalar,
                        (1, 0): nc.sync, (1, 1): nc.gpsimd}
            for m in range(2):
                for hh in range(ET // ETH):
                    ktg_engs[(m, hh)].dma_start(
                        kt_sb[:, hh * ETH:(hh + 1) * ETH,
                              m * SQ:(m + 1) * SQ],
                        kt_g_d[:][m * D + hh * ETH * P:
                                  m * D + (hh + 1) * ETH * P, :].rearrange(
                            "(t p) k -> p t k", p=P))
            vg_v = v_g_d[:].rearrange("(t p) e -> p t e", p=P)
            nc.sync.dma_start(v[:, :, 0:D], vg_v)

            # Q projection last: overlaps the collectives
            for et in range(ET):
                for qo, qn in _chunks(SQ, 512):
                    ps = proj(wq, xqt, qo, qn, et)
                    nc.vector.tensor_copy(qt[:, et, qo:qo + qn], ps[:, :qn])

        # ---- scores^T + exp: expT[k, q] = exp(KT.T @ QT + mask_bias[k]) ----
        wo = load_w(wo_d, nc.gpsimd)  # prefetch for the output projection
        acc = con.tile([P, SQ], f32)
        nc.gpsimd.memset(acc[:], 0.0)
        ones128 = con.tile([P, 1], f32)
        nc.gpsimd.memset(ones128[:], 1.0)
        expt = bigA.tile([P, KT, SQ], mmdt, tag="bigA")
        for kt_ in range(KT):
            for qo, qn in _chunks(SQ, 512):
                ps = pp.tile([P, 512], f32, tag="pp")
                for et in range(ET):
                    nc.tensor.matmul(
                        ps[:, :qn],
                        kt_sb[:, et, kt_ * P:(kt_ + 1) * P],
                        qt[:, et, qo:qo + qn],
                        start=(et == 0),
                        stop=(et == ET - 1),
                    )
                nc.scalar.activation(
                    expt[:, kt_, qo:qo + qn], ps[:, :qn], Exp,
                    bias=mb[:, kt_:kt_ + 1], scale=1.0,
                )
                # partial row-sums on the otherwise-idle DVE: acc[p, q] =
                # sum_kt expT[p, kt, q]; the final partition reduction is
                # then 2 matmuls instead of 32 full-stream ones-row matmuls
                nc.vector.tensor_add(acc[:, qo:qo + qn], acc[:, qo:qo + qn],
                                     expt[:, kt_, qo:qo + qn])

        # ---- O pass: O^T_unnorm[e, q] (+ rsum[1, q]) = V.T @ expT ----
        # One PSUM bank per accumulation group: each (m, q-chunk) gets its
        # own [P, 512] bank, accumulated over all key tiles.
        ot = bigA.tile([P, ET, SQ], mmdt, tag="bigA")
        rsum_sb = con.tile([1, SQ], f32)
        for qo, qn in _chunks(SQ, 512):
            for m in range(ET):
                ps = pp.tile([P, 512], f32, tag="pp")
                for kt_ in range(KT):
                    nc.tensor.matmul(
                        ps[:, :qn], v[:, kt_, m * P:(m + 1) * P],
                        expt[:, kt_, qo:qo + qn],
                        start=(kt_ == 0), stop=(kt_ == KT - 1),
                    )
                nc.vector.tensor_copy(ot[:, m, qo:qo + qn], ps[:, :qn])
            rs = rsp.tile([1, 512], f32, tag="rs")
            nc.tensor.matmul(rs[:, :qn], ones128[:], acc[:, qo:qo + qn],
                             start=True, stop=True)
            nc.scalar.copy(rsum_sb[:, qo:qo + qn], rs[:, :qn])

        # ---- 1/rsum as per-partition scalars: transpose [1, SQ] -> [P, QT] ----
        # All QT column-writes form one accumulation group (disjoint columns
        # of a single bank; start would lazily re-zero the whole bank).
        rsT = rsp.tile([P, QT], f32, tag="rsT")
        for t in range(QT):
            nc.tensor.matmul(
                rsT[:, t:t + 1], rsum_sb[:, t * P:(t + 1) * P], ones1[:],
                start=(t == 0), stop=(t == QT - 1),
            )
        rinv = con.tile([P, QT], f32)
        nc.vector.reciprocal(rinv[:], rsT[:])

        # ---- output projection + normalize + residual ----
        hs_v = hs_d.ap().rearrange("(t p) f -> t p f", p=P)
        out_v = out_d.ap().rearrange("(t p) f -> t p f", p=P)
        out_engs = [nc.sync, nc.scalar, nc.gpsimd]
        for qt_ in range(QT):
            hst = outp.tile([P, D], f32, tag="hst")
            nc.scalar.dma_start(hst[:], hs_v[qt_])
            outt = outp.tile([P, D], f32, tag="outt")
            for ci, (fo, fn) in enumerate(_chunks(D, 512)):
                ps = pp.tile([P, 512], f32, tag="pp")
                for et in range(ET):
                    nc.tensor.matmul(
                        ps[:, :fn],
                        ot[:, et, qt_ * P:(qt_ + 1) * P],
                        wo[:, et, fo:fo + fn],
                        start=(et == 0),
                        stop=(et == ET - 1),
                    )
                nc.vector.scalar_tensor_tensor(
                    outt[:, fo:fo + fn], ps[:, :fn], rinv[:, qt_:qt_ + 1],
                    hst[:, fo:fo + fn], op0=mult, op1=add,
                )
                out_engs[(qt_ * 2 + ci) % 3].dma_start(
                    out_v[qt_][:, fo:fo + fn], outt[:, fo:fo + fn])

    nc.compile()
    return nc


DEDUP = True


@functools.lru_cache(maxsize=2)
def _get_program(D, SQ, SKV, dedup=DEDUP):
    return build_program(D, SQ, SKV, dedup=dedup)


def _numpy_reference(hidden_states, mask, Wq, bq, Wk, bk, Wv, bv, Wo, bo):
    """Exact fallback (used only if bq/bk are nonzero, which the spec excludes)."""
    x = hidden_states.astype(np.float64)
    q = x @ Wq.T.astype(np.float64) + bq
    k = x @ Wk.T.astype(np.float64) + bk
    v = x @ Wv.T.astype(np.float64) + bv
    s = np.einsum("bqd,bkd->bqk", q, k) / np.sqrt(x.shape[-1])
    s = np.where(mask[:, None, :] == 0, -1e10, s)
    s -= s.max(axis=-1, keepdims=True)
    e = np.exp(s)
    a = e / e.sum(axis=-1, keepdims=True)
    hid = np.einsum("bqk,bkd->bqd", a, v)
    out = x + hid @ Wo.T.astype(np.float64) + bo
    return out.astype(np.float32)


def make_in_maps(hidden_states, mask, Wq, bq, Wk, bk, Wv, bv, Wo, bo):
    hs = np.asarray(hidden_states, dtype=np.float32)
    mask = np.asarray(mask)
    B, S, D = hs.shape
    SQ = S // 2
    scale = np.float32(float(int(D) ** (-0.5)))

    wq_h = np.ascontiguousarray(np.asarray(Wq, np.float32).T * scale).astype(np.float16)
    wk_h = np.ascontiguousarray(np.asarray(Wk, np.float32).T).astype(np.float16)
    wv_h = np.ascontiguousarray(np.asarray(Wv, np.float32).T).astype(np.float16)
    wo_h = np.ascontiguousarray(np.asarray(Wo, np.float32).T).astype(np.float16)
    # v-bias and o-bias act as a constant shift after the output projection:
    # fold them into the residual input (exact).
    extra = (np.asarray(Wo, np.float32) @ np.asarray(bv, np.float32)
             + np.asarray(bo, np.float32))

    in_maps = []
    for c in range(N_CORES):
        b, h = divmod(c, 2)
        xb = hs[b]
        xqT = np.ascontiguousarray(xb[h * SQ:(h + 1) * SQ].T.astype(np.float16))
        hsc = np.ascontiguousarray(xb[h * SQ:(h + 1) * SQ] + extra[None, :])
        mb = np.where(mask[b] == 0, np.float32(NEG_BIAS), np.float32(0.0))
        mb = np.ascontiguousarray(mb.reshape(S // P, P).T.astype(np.float32))
        m = dict(xqt=xqT, hs=hsc, wq=wq_h, wk=wk_h, wv=wv_h, wo=wo_h, mb=mb)
        if not DEDUP:
            m["xt"] = np.ascontiguousarray(xb.T.astype(np.float16))
        in_maps.append(m)
    return in_maps


def assemble_output(results, B, S, D):
    SQ = S // 2
    out = np.empty((B, S, D), np.float32)
    for c in range(N_CORES):
        b, h = divmod(c, 2)
        out[b, h * SQ:(h + 1) * SQ, :] = results[c]["out"]
    return out


def kernel(hidden_states, mask, Wq, bq, Wk, bk, Wv, bv, Wo, bo):
    hs = np.asarray(hidden_states, dtype=np.float32)
    B, S, D = hs.shape
    args = dict(hidden_states=hs, mask=np.asarray(mask),
                Wq=np.asarray(Wq, np.float32), bq=np.asarray(bq, np.float32),
                Wk=np.asarray(Wk, np.float32), bk=np.asarray(bk, np.float32),
                Wv=np.asarray(Wv, np.float32), bv=np.asarray(bv, np.float32),
                Wo=np.asarray(Wo, np.float32), bo=np.asarray(bo, np.float32))
    if np.any(args["bq"]) or np.any(args["bk"]):
        return _numpy_reference(**args)

    nc = _get_program(D, S // 2, S)
    in_maps = make_in_maps(**args)
    res = run_bass_kernel_spmd(nc, in_maps, core_ids=list(range(N_CORES)))
    return assemble_output(res.results, B, S, D)


if __name__ == "__main__":
    rng = np.random.default_rng(0)
    B, S, D = 4, 2048, 1024
    ins = dict(
        hidden_states=rng.standard_normal((B, S, D), np.float32),
        mask=rng.integers(0, 2, (B, S)).astype(np.int32),
        Wq=rng.standard_normal((D, D), np.float32) / np.sqrt(D),
        bq=np.zeros(D, np.float32),
        Wk=rng.standard_normal((D, D), np.float32) / np.sqrt(D),
        bk=np.zeros(D, np.float32),
        Wv=rng.standard_normal((D, D), np.float32) / np.sqrt(D),
        bv=np.zeros(D, np.float32),
        Wo=rng.standard_normal((D, D), np.float32) / np.sqrt(D),
        bo=np.zeros(D, np.float32),
    )
    out = kernel(**ins)
    ref = _numpy_reference(**ins)
    err = np.max(np.abs(out - ref)) / np.max(np.abs(ref))
    print("rel err vs numpy:", err)



# revision 3
# speedup vs baseline: 1.6783x; 1.6783x over previous
"""Trainium2 Bass kernel for a single-head AttentionBlock with residual.

Reference computation (per batch b):
    q = x @ Wq^T ; k = x @ Wk^T ; v = x @ Wv^T      (bq/bk/bv zero per spec)
    s = (q @ k^T) / sqrt(D)         [S, S]
    s = where(mask[b] == 0 (keys), -1e10, s)
    a = softmax(s, axis=-1)
    out = x + (a @ v) @ Wo^T + bo

Sharding: 8 cores = 4 batches x 2 query-halves (SQ=1024 rows each).

Two optimizations over the fp16 dense baseline:
 1. fp8 (e4m3) matmuls in DoubleRow perf mode: each matmul consumes two
    128-row contraction subtiles at once (2x PE throughput vs fp16).
    Scale bookkeeping: weights are pre-scaled x32 on the host so their
    entries sit in fp8's normal range; Q/K are kept raw (std ~32), V is
    rescaled /32 at the psum->fp8 cast, scores get exp(2^-15 * ps + mb)
    where mb also carries -6*ln2 so expt = 2^-6 * exp(s) stays in fp8
    range through the A@V accumulation.
 2. Masked-key compaction: mask[b] knocks out ~half the keys; the host
    gathers each core's unmasked local keys (<=538 of 1024 for the spec
    inputs) into a padded [D, KH=576] block. K/V are projected only for
    those, the pair of cores exchanges halves via a 2-member AllGather
    (DRAM bounce), and scores/A@V run over 2*KH=1152 key slots (9 tiles)
    instead of 2048 (16 tiles). Pad slots get bias -30000 -> exp == 0.

Row sums ride along in the A@V pass via a ones column appended to V
(lhsT [128,2,1] DoubleRow matmuls into a [1, q] psum), so the vector
engine only does psum->fp8 casts and the final normalize+residual.

Softmax max-subtraction is skipped: scores are ~N(0,1) here, exp < ~200,
and the 2^-6 rescale keeps everything comfortably inside fp8/fp32.

bq/bk are assumed zero (spec fill=zeros); nonzero or a mask half-count
above KH triggers an exact numpy fallback (never hit for the spec
inputs). bv/bo are folded into the residual on the host (exact).
"""

import functools
from contextlib import ExitStack

import numpy as np
import ml_dtypes

import concourse.bass as bass
import concourse.tile as tile
from concourse import bacc, mybir
from concourse.bass_utils import run_bass_kernel_spmd

P = 128
NEG_BIAS = -30000.0
N_CORES = 8
KH = 576                 # per-half compacted key capacity (4.5 tiles)
WSCALE = 32.0            # host pre-scale on all weight matrices
EXP_OFF = -6.0 * float(np.log(2.0))   # expt = 2^-6 * exp(s)
FP8 = ml_dtypes.float8_e4m3fn


def _chunks(total, size):
    return [(o, min(size, total - o)) for o in range(0, total, size)]


def build_program(D=1024, SQ=1024, kh=KH, n_cores=8):
    """Build + compile the single-core Bass program (same program on all cores)."""
    f32 = mybir.dt.float32
    fp8 = mybir.dt.float8e4
    DT = D // P    # d contraction tiles
    ET = D // P    # e tiles
    KTc = (2 * kh) // P   # gathered key tiles (9)
    QT = SQ // P   # query row tiles
    DR = mybir.MatmulPerfMode.DoubleRow

    nc = bacc.Bacc("TRN2", target_bir_lowering=False, debug=False,
                   num_devices=n_cores)

    xqt_d = nc.dram_tensor("xqt", [D, SQ], fp8, kind="ExternalInput")
    xkt_d = nc.dram_tensor("xkt", [D, kh], fp8, kind="ExternalInput")
    hs_d = nc.dram_tensor("hs", [SQ, D], f32, kind="ExternalInput")
    wq_d = nc.dram_tensor("wq", [D, D], fp8, kind="ExternalInput")
    wk_d = nc.dram_tensor("wk", [D, D], fp8, kind="ExternalInput")
    wv_d = nc.dram_tensor("wv", [D, D], fp8, kind="ExternalInput")
    wo_d = nc.dram_tensor("wo", [D, D], fp8, kind="ExternalInput")
    mb_d = nc.dram_tensor("mb", [P, KTc], f32, kind="ExternalInput")
    out_d = nc.dram_tensor("out", [SQ, D], f32, kind="ExternalOutput")

    Exp = mybir.ActivationFunctionType.Exp
    Copy = mybir.ActivationFunctionType.Copy
    mult = mybir.AluOpType.mult
    add = mybir.AluOpType.add

    with tile.TileContext(nc) as tc, ExitStack() as ctx:
        bigA = ctx.enter_context(tc.tile_pool(name="bigA", bufs=2))
        qk_pool = ctx.enter_context(tc.tile_pool(name="qk", bufs=1))
        v_pool = ctx.enter_context(tc.tile_pool(name="vp", bufs=1))
        wpool = ctx.enter_context(tc.tile_pool(name="w", bufs=2))
        con = ctx.enter_context(tc.tile_pool(name="const", bufs=1))
        outp = ctx.enter_context(tc.tile_pool(name="outs", bufs=2))

        pp = ctx.enter_context(tc.tile_pool(name="pp", bufs=5, space="PSUM"))
        rsp = ctx.enter_context(tc.tile_pool(name="rsp", bufs=1, space="PSUM"))

        # ---- PE warmup during the initial DMA wait (HAM ramp) ----
        ones1h = con.tile([1, 1], fp8)
        nc.gpsimd.memset(ones1h[:], 1.0)
        warm_in = con.tile([1, 256], fp8)
        nc.gpsimd.memset(warm_in[:], 0.0)
        warm_ps = pp.tile([P, 512], f32, tag="pp")
        N_WARM = 16
        for i in range(N_WARM):
            nc.tensor.matmul(warm_ps[:1, :256], ones1h[:], warm_in[:],
                             start=(i == 0), stop=(i == N_WARM - 1))
        warm_out = con.tile([1, 256], f32)
        nc.vector.tensor_copy(warm_out[:], warm_ps[:1, :256])

        mb = con.tile([P, KTc], f32)
        nc.gpsimd.dma_start(mb[:], mb_d.ap())
        ones1 = con.tile([1, 1], f32)
        nc.gpsimd.memset(ones1[:], WSCALE)  # rsT = 32*rsum -> rinv = 2/Z

        # ---- loads split across the three DMA-capable queues ----
        _engs = [nc.gpsimd, nc.sync, nc.scalar]

        def load_w(dram, eng=None, split=1):
            w = wpool.tile([P, DT, D], fp8, tag="w")
            wv_ = dram.ap().rearrange("(t p) e -> p t e", p=P)
            split = min(split, DT)
            step = DT // split
            for i in range(split):
                e = _engs[i % 3] if eng is None else eng
                sl = slice(i * step, (i + 1) * step)
                e.dma_start(w[:, sl, :], wv_[:, sl, :])
            return w

        def projDR(w, x, xo, xn, et, psn=512):
            """psum <- w[:, :, et].T @ x[:, :, xo:xo+xn], DoubleRow pairs."""
            ps = pp.tile([P, psn], f32, tag="pp")
            for t in range(DT // 2):
                nc.tensor.matmul(
                    ps[:, :xn], w[:, 2 * t:2 * t + 2, et * P:(et + 1) * P],
                    x[:, 2 * t:2 * t + 2, xo:xo + xn],
                    start=(t == 0), stop=(t == DT // 2 - 1),
                    perf_mode=DR,
                )
            return ps

        kt_sb = qk_pool.tile([P, ET, 2 * kh], fp8, tag="kt")
        # free width D+16 keeps the DoubleRow pair-dim stride 16B-aligned
        # (dual-fp8 Ldweights ISA restriction); col D is the ones column.
        v = v_pool.tile([P, KTc, D + 16], fp8, tag="v")
        nc.gpsimd.memset(v[:, :, D:D + 1], 1.0)  # ones col -> row sums
        qt = qk_pool.tile([P, ET, SQ], fp8, tag="qt")

        pairs = [[2 * b, 2 * b + 1] for b in range(n_cores // 2)]
        dram = ctx.enter_context(tc.tile_pool(name="dram", bufs=1, space="DRAM"))
        stg = ctx.enter_context(tc.tile_pool(name="stg", bufs=6))

        # CC-core warmup: absorb the ~70us boot-once dispatch latency of the
        # collectives core so the real K/V exchanges dispatch immediately.
        ccw_in_d = dram.tile([P, 16], f32, tag="ccwi", name="ccw_in")
        ccw_out_d = dram.tile([2 * P, 16], f32, tag="ccwo", name="ccw_out")
        ccw_sb = con.tile([P, 16], f32)
        nc.gpsimd.memset(ccw_sb[:], 0.0)
        nc.sync.dma_start(ccw_in_d[:], ccw_sb[:])
        nc.gpsimd.collective_compute(
            "AllGather", mybir.AluOpType.bypass, replica_groups=pairs,
            ins=[ccw_in_d[:].opt()], outs=[ccw_out_d[:].opt()],
        )

        kt_loc_d = dram.tile([D, kh], fp8, tag="ktl", name="kt_loc")
        kt_g_d = dram.tile([2 * D, kh], fp8, tag="ktg", name="kt_g")
        v_loc_d = dram.tile([kh, D], fp8, tag="vl")
        v_g_d = dram.tile([2 * kh, D], fp8, tag="vg")

        xkt = con.tile([P, DT, kh], fp8)
        xkt_v = xkt_d.ap().rearrange("(t p) k -> p t k", p=P)
        wk = load_w(wk_d, split=4)
        for i in range(2):
            sl = slice(i * (DT // 2), (i + 1) * (DT // 2))
            _engs[(i + 1) % 3].dma_start(xkt[:, sl, :], xkt_v[:, sl, :])
        wv = load_w(wv_d, nc.gpsimd)
        xqt = bigA.tile([P, DT, SQ], fp8, tag="bigA")
        xqt_v = xqt_d.ap().rearrange("(t p) q -> p t q", p=P)
        for i in range(4):
            sl = slice(i * (DT // 4), (i + 1) * (DT // 4))
            _engs[i % 3].dma_start(xqt[:, sl, :], xqt_v[:, sl, :])
        wq = load_w(wq_d, nc.gpsimd)

        # K_loc[e, k_loc] = wkT.T @ xkt -> SBUF staging -> DRAM bounce
        ktl_v = kt_loc_d[:].rearrange("(t p) k -> p t k", p=P)
        si = 0
        for et in range(ET):
            for ko, kn in _chunks(kh, 512):
                ps = projDR(wk, xkt, ko, kn, et)
                st = stg.tile([P, 512], fp8, tag="stage")
                nc.vector.tensor_copy(st[:, :kn], ps[:, :kn])
                (nc.sync if si % 2 else nc.scalar).dma_start(
                    ktl_v[:, et, ko:ko + kn], st[:, :kn])
                si += 1
        nc.gpsimd.collective_compute(
            "AllGather", mybir.AluOpType.bypass, replica_groups=pairs,
            ins=[kt_loc_d[:].opt()], outs=[kt_g_d[:].opt()],
        )

        # V_loc[k_loc, e] = (xkt.T @ wvT) / 32 -> SBUF staging -> DRAM bounce
        vl_v = v_loc_d[:].rearrange("k e -> k e")
        for vo, vn in _chunks(kh, P):
            for eo, en in _chunks(D, 512):
                ps = pp.tile([P, 512], f32, tag="pp")
                for t in range(DT // 2):
                    nc.tensor.matmul(
                        ps[:vn, :en], xkt[:, 2 * t:2 * t + 2, vo:vo + vn],
                        wv[:, 2 * t:2 * t + 2, eo:eo + en],
                        start=(t == 0), stop=(t == DT // 2 - 1),
                        perf_mode=DR,
                    )
                st = stg.tile([P, 512], fp8, tag="stage")
                nc.scalar.activation(st[:vn, :en], ps[:vn, :en], Copy,
                                     bias=0.0, scale=1.0 / WSCALE)
                (nc.sync if si % 2 else nc.scalar).dma_start(
                    vl_v[vo:vo + vn, eo:eo + en], st[:vn, :en])
                si += 1
        nc.gpsimd.collective_compute(
            "AllGather", mybir.AluOpType.bypass, replica_groups=pairs,
            ins=[v_loc_d[:].opt()], outs=[v_g_d[:].opt()],
        )
        # K gather-ins (emitted after the V stage-outs: queue order is
        # emission order; a big gather parked ahead of the staging DMAs
        # would stall slot recycling). 4-way split across all queues.
        ETH = ET // 2
        ktg_engs = {(0, 0): nc.gpsimd, (0, 1): nc.scalar,
                    (1, 0): nc.sync, (1, 1): nc.gpsimd}
        for m in range(2):
            for hh in range(2):
                ktg_engs[(m, hh)].dma_start(
                    kt_sb[:, hh * ETH:(hh + 1) * ETH,
                          m * kh:(m + 1) * kh],
                    kt_g_d[:][m * D + hh * ETH * P:
                              m * D + (hh + 1) * ETH * P, :].rearrange(
                        "(t p) k -> p t k", p=P))
        vg_v = v_g_d[:].rearrange("(t p) e -> p t e", p=P)
        nc.sync.dma_start(v[:, :, 0:D], vg_v)

        # Q projection last: overlaps the collectives
        for et in range(ET):
            for qo, qn in _chunks(SQ, 512):
                ps = projDR(wq, xqt, qo, qn, et)
                nc.vector.tensor_copy(qt[:, et, qo:qo + qn], ps[:, :qn])

        # ---- scores^T + exp: expT[k, q] = 2^-6 exp(KT.T @ QT * 2^-15 + mask) ----
        wo = load_w(wo_d, nc.gpsimd)  # prefetch for the output projection
        expt = bigA.tile([P, KTc, SQ], fp8, tag="bigA")
        for kt_ in range(KTc):
            for qo, qn in _chunks(SQ, 512):
                ps = pp.tile([P, 512], f32, tag="pp")
                for t in range(ET // 2):
                    nc.tensor.matmul(
                        ps[:, :qn],
                        kt_sb[:, 2 * t:2 * t + 2, kt_ * P:(kt_ + 1) * P],
                        qt[:, 2 * t:2 * t + 2, qo:qo + qn],
                        start=(t == 0), stop=(t == ET // 2 - 1),
                        perf_mode=DR,
                    )
                nc.scalar.activation(
                    expt[:, kt_, qo:qo + qn], ps[:, :qn], Exp,
                    bias=mb[:, kt_:kt_ + 1], scale=float(2.0 ** -15),
                )

        # ---- O pass: O^T_unnorm[e, q] (+ rsum[1, q]) = V.T @ expT ----
        ot = bigA.tile([P, ET, SQ], fp8, tag="bigA")
        rsum_sb = con.tile([1, SQ], f32)
        KP = KTc // 2  # DoubleRow pairs; KTc odd -> one single tile at the end

        def av_group(lhs_lo, lhs_n, ps, qo, qn):
            for t in range(KP):
                nc.tensor.matmul(
                    ps[:lhs_n, :qn],
                    v[:, 2 * t:2 * t + 2, lhs_lo:lhs_lo + lhs_n],
                    expt[:, 2 * t:2 * t + 2, qo:qo + qn],
                    start=(t == 0), stop=False, perf_mode=DR,
                )
            nc.tensor.matmul(
                ps[:lhs_n, :qn], v[:, 2 * KP, lhs_lo:lhs_lo + lhs_n],
                expt[:, 2 * KP, qo:qo + qn],
                start=False, stop=True,
            )

        for qo, qn in _chunks(SQ, 512):
            for m in range(ET):
                ps = pp.tile([P, 512], f32, tag="pp")
                av_group(m * P, P, ps, qo, qn)
                nc.vector.tensor_copy(ot[:, m, qo:qo + qn], ps[:, :qn])
            rs = rsp.tile([1, 512], f32, tag="rs")
            av_group(D, 1, rs, qo, qn)
            nc.scalar.copy(rsum_sb[:, qo:qo + qn], rs[:, :qn])

        # ---- 1/rsum as per-partition scalars: transpose [1, SQ] -> [P, QT] ----
        rsT = rsp.tile([P, QT], f32, tag="rsT")
        for t in range(QT):
            nc.tensor.matmul(
                rsT[:, t:t + 1], rsum_sb[:, t * P:(t + 1) * P], ones1[:],
                start=(t == 0), stop=(t == QT - 1),
            )
        rinv = con.tile([P, QT], f32)
        nc.vector.reciprocal(rinv[:], rsT[:])

        # ---- output projection + normalize + residual ----
        hs_v = hs_d.ap().rearrange("(t p) f -> t p f", p=P)
        out_v = out_d.ap().rearrange("(t p) f -> t p f", p=P)
        out_engs = [nc.sync, nc.scalar, nc.gpsimd]
        for qt_ in range(QT):
            hst = outp.tile([P, D], f32, tag="hst")
            nc.scalar.dma_start(hst[:], hs_v[qt_])
            outt = outp.tile([P, D], f32, tag="outt")
            for ci, (fo, fn) in enumerate(_chunks(D, 512)):
                ps = pp.tile([P, 512], f32, tag="pp")
                for t in range(ET // 2):
                    nc.tensor.matmul(
                        ps[:, :fn],
                        ot[:, 2 * t:2 * t + 2, qt_ * P:(qt_ + 1) * P],
                        wo[:, 2 * t:2 * t + 2, fo:fo + fn],
                        start=(t == 0), stop=(t == ET // 2 - 1),
                        perf_mode=DR,
                    )
                nc.vector.scalar_tensor_tensor(
                    outt[:, fo:fo + fn], ps[:, :fn], rinv[:, qt_:qt_ + 1],
                    hst[:, fo:fo + fn], op0=mult, op1=add,
                )
                out_engs[(qt_ * 2 + ci) % 3].dma_start(
                    out_v[qt_][:, fo:fo + fn], outt[:, fo:fo + fn])

    nc.compile()
    return nc


@functools.lru_cache(maxsize=2)
def _get_program(D, SQ):
    return build_program(D, SQ)


def _numpy_reference(hidden_states, mask, Wq, bq, Wk, bk, Wv, bv, Wo, bo):
    """Exact fallback (used only if bq/bk nonzero or mask counts exceed KH)."""
    x = hidden_states.astype(np.float64)
    q = x @ Wq.T.astype(np.float64) + bq
    k = x @ Wk.T.astype(np.float64) + bk
    v = x @ Wv.T.astype(np.float64) + bv
    s = np.einsum("bqd,bkd->bqk", q, k) / np.sqrt(x.shape[-1])
    s = np.where(mask[:, None, :] == 0, -1e10, s)
    s -= s.max(axis=-1, keepdims=True)
    e = np.exp(s)
    a = e / e.sum(axis=-1, keepdims=True)
    hid = np.einsum("bqk,bkd->bqd", a, v)
    out = x + hid @ Wo.T.astype(np.float64) + bo
    return out.astype(np.float32)


def make_in_maps(hidden_states, mask, Wq, bq, Wk, bk, Wv, bv, Wo, bo):
    hs = np.asarray(hidden_states, dtype=np.float32)
    mask = np.asarray(mask)
    B, S, D = hs.shape
    SQ = S // 2
    KTc = (2 * KH) // P

    wq8 = np.ascontiguousarray(np.asarray(Wq, np.float32).T * WSCALE).astype(FP8)
    wk8 = np.ascontiguousarray(np.asarray(Wk, np.float32).T * WSCALE).astype(FP8)
    wv8 = np.ascontiguousarray(np.asarray(Wv, np.float32).T * WSCALE).astype(FP8)
    wo8 = np.ascontiguousarray(np.asarray(Wo, np.float32).T * WSCALE).astype(FP8)
    # v-bias and o-bias act as a constant shift after the output projection:
    # fold them into the residual input (exact).
    extra = (np.asarray(Wo, np.float32) @ np.asarray(bv, np.float32)
             + np.asarray(bo, np.float32))

    # per-(batch,half) compacted key indices
    idxs = {}
    for b in range(B):
        for h in range(2):
            idx = np.nonzero(mask[b, h * SQ:(h + 1) * SQ])[0]
            if len(idx) > KH:
                return None  # caller falls back to numpy
            idxs[(b, h)] = idx

    in_maps = []
    for c in range(N_CORES):
        b, h = divmod(c, 2)
        xb = hs[b]
        x8 = xb.astype(FP8)
        xqT = np.ascontiguousarray(x8[h * SQ:(h + 1) * SQ].T)
        idx = idxs[(b, h)]
        xkT = np.zeros((D, KH), FP8)
        xkT[:, :len(idx)] = x8[h * SQ + idx].T
        hsc = np.ascontiguousarray(xb[h * SQ:(h + 1) * SQ] + extra[None, :])
        # gathered-key bias: slot k valid iff k%KH < count(member); pad/masked
        # slots -30000; -6ln2 everywhere for the 2^-6 exp prescale.
        bias = np.full(2 * KH, np.float32(NEG_BIAS))
        for m in range(2):
            cnt = len(idxs[(b, m)])
            bias[m * KH:m * KH + cnt] = 0.0
        bias += np.float32(EXP_OFF)
        mb = np.ascontiguousarray(
            bias.reshape(KTc, P).T.astype(np.float32))
        in_maps.append(dict(xqt=xqT, xkt=xkT, hs=hsc, wq=wq8, wk=wk8,
                            wv=wv8, wo=wo8, mb=mb))
    return in_maps


def assemble_output(results, B, S, D):
    SQ = S // 2
    out = np.empty((B, S, D), np.float32)
    for c in range(N_CORES):
        b, h = divmod(c, 2)
        out[b, h * SQ:(h + 1) * SQ, :] = results[c]["out"]
    return out


def kernel(hidden_states, mask, Wq, bq, Wk, bk, Wv, bv, Wo, bo):
    hs = np.asarray(hidden_states, dtype=np.float32)
    B, S, D = hs.shape
    args = dict(hidden_states=hs, mask=np.asarray(mask),
                Wq=np.asarray(Wq, np.float32), bq=np.asarray(bq, np.float32),
                Wk=np.asarray(Wk, np.float32), bk=np.asarray(bk, np.float32),
                Wv=np.asarray(Wv, np.float32), bv=np.asarray(bv, np.float32),
                Wo=np.asarray(Wo, np.float32), bo=np.asarray(bo, np.float32))
    if np.any(args["bq"]) or np.any(args["bk"]) or (S, D) != (2048, 1024):
        return _numpy_reference(**args)

    in_maps = make_in_maps(**args)
    if in_maps is None:
        return _numpy_reference(**args)
    nc = _get_program(D, S // 2)
    res = run_bass_kernel_spmd(nc, in_maps, core_ids=list(range(N_CORES)))
    return assemble_output(res.results, B, S, D)


if __name__ == "__main__":
    rng = np.random.default_rng(0)
    B, S, D = 4, 2048, 1024
    ins = dict(
        hidden_states=rng.standard_normal((B, S, D)).astype(np.float32),
        mask=rng.integers(0, 2, (B, S)).astype(np.int32),
        Wq=(rng.standard_normal((D, D)) / np.sqrt(D)).astype(np.float32),
        bq=np.zeros(D, np.float32),
        Wk=(rng.standard_normal((D, D)) / np.sqrt(D)).astype(np.float32),
        bk=np.zeros(D, np.float32),
        Wv=(rng.standard_normal((D, D)) / np.sqrt(D)).astype(np.float32),
        bv=np.zeros(D, np.float32),
        Wo=(rng.standard_normal((D, D)) / np.sqrt(D)).astype(np.float32),
        bo=np.zeros(D, np.float32),
    )
    out = kernel(**ins)
    ref = _numpy_reference(**ins)
    err = np.max(np.abs(out - ref)) / np.max(np.abs(ref))
    print("rel err vs numpy:", err)


# revision 6
# speedup vs baseline: 1.7543x; 1.0453x over previous
"""Trainium2 Bass kernel for a single-head AttentionBlock with residual.

Reference computation (per batch b):
    q = x @ Wq^T ; k = x @ Wk^T ; v = x @ Wv^T      (bq/bk/bv zero per spec)
    s = (q @ k^T) / sqrt(D)         [S, S]
    s = where(mask[b] == 0 (keys), -1e10, s)
    a = softmax(s, axis=-1)
    out = x + (a @ v) @ Wo^T + bo

Sharding: 8 cores = 4 batches x 2 query-halves (SQ=1024 rows each).

Optimizations over the fp16 dense baseline:
 1. fp8 (e4m3) matmuls in DoubleRow perf mode: each matmul consumes two
    128-row contraction subtiles at once (2x PE throughput vs fp16).
    Scale bookkeeping: weights are pre-scaled x32 on the host so their
    entries sit in fp8's normal range; Q/K are kept raw (std ~32), V is
    rescaled /32 at the psum->fp8 cast, scores get exp(2^-15 * ps + mb)
    where mb also carries -6*ln2 so expt = 2^-6 * exp(s) stays in fp8
    range through the A@V accumulation.  (Dual-fp8 Ldweights requires
    the pair-dim byte stride to be 16B-aligned -> V is padded to D+16.)
 2. Masked-key compaction: mask[b] knocks out ~half the keys; the host
    gathers each core's unmasked local keys (<=538 of 1024 for the spec
    inputs) into a padded [D, KH=576] block. K/V are projected only for
    those, the pair of cores exchanges halves via a 2-member AllGather
    (DRAM bounce), and scores/A@V run over 2*KH=1152 key slots (9 tiles)
    instead of 2048 (16 tiles). Pad slots get bias -30000 -> exp == 0.
 3. Schedule: the CC warmup AllGather is triggered as the very first
    instruction (the collectives core takes ~30us to boot after its
    first trigger); K is projected+staged first so its exchange rides
    right behind the warmup; V exchange overlaps Q projection; compute
    runs chunk-outer (scores -> A@V+rsum -> out-projection per 512-query
    chunk) so the final stores drain while the other chunk computes.

Row sums ride along in the A@V pass via a ones column appended to V
(lhsT [128,2,1] DoubleRow matmuls into a [1, q] psum), so the vector
engine only does psum->fp8 casts and the final normalize+residual.

Softmax max-subtraction is skipped: scores are ~N(0,1) here, exp < ~200,
and the 2^-6 rescale keeps everything comfortably inside fp8/fp32.

bq/bk are assumed zero (spec fill=zeros); nonzero or a mask half-count
above KH triggers an exact numpy fallback (never hit for the spec
inputs). bv/bo are folded into the residual on the host (exact).
"""

import functools
from contextlib import ExitStack

import numpy as np
import ml_dtypes

import concourse.bass as bass
import concourse.tile as tile
from concourse import bacc, mybir
from concourse.bass_utils import run_bass_kernel_spmd

P = 128
NEG_BIAS = -30000.0
N_CORES = 8
KH = 576                 # per-half compacted key capacity (4.5 tiles)
WSCALE = 32.0            # host pre-scale on all weight matrices
EXP_OFF = -6.0 * float(np.log(2.0))   # expt = 2^-6 * exp(s)
FP8 = ml_dtypes.float8_e4m3fn


def _chunks(total, size):
    return [(o, min(size, total - o)) for o in range(0, total, size)]


def build_program(D=1024, SQ=1024, kh=KH, n_cores=8):
    """Build + compile the single-core Bass program (same program on all cores)."""
    f32 = mybir.dt.float32
    f16 = mybir.dt.float16
    fp8 = mybir.dt.float8e4
    DT = D // P    # d contraction tiles
    ET = D // P    # e tiles
    KTc = (2 * kh) // P   # gathered key tiles (9)
    QT = SQ // P   # query row tiles
    DR = mybir.MatmulPerfMode.DoubleRow

    nc = bacc.Bacc("TRN2", target_bir_lowering=False, debug=False,
                   num_devices=n_cores)

    xqt_d = nc.dram_tensor("xqt", [D, SQ], fp8, kind="ExternalInput")
    xkt_d = nc.dram_tensor("xkt", [D, kh], fp8, kind="ExternalInput")
    hs_d = nc.dram_tensor("hs", [SQ, D], f16, kind="ExternalInput")
    wq_d = nc.dram_tensor("wq", [D, D], fp8, kind="ExternalInput")
    wk_d = nc.dram_tensor("wk", [D, D], fp8, kind="ExternalInput")
    wv_d = nc.dram_tensor("wv", [D, D], fp8, kind="ExternalInput")
    wo_d = nc.dram_tensor("wo", [D, D], fp8, kind="ExternalInput")
    mb_d = nc.dram_tensor("mb", [P, KTc], f32, kind="ExternalInput")
    out_d = nc.dram_tensor("out", [SQ, D], f32, kind="ExternalOutput")

    Exp = mybir.ActivationFunctionType.Exp
    Copy = mybir.ActivationFunctionType.Copy
    mult = mybir.AluOpType.mult
    add = mybir.AluOpType.add

    with tile.TileContext(nc) as tc, ExitStack() as ctx:
        bigA = ctx.enter_context(tc.tile_pool(name="bigA", bufs=2))
        qk_pool = ctx.enter_context(tc.tile_pool(name="qk", bufs=1))
        v_pool = ctx.enter_context(tc.tile_pool(name="vp", bufs=1))
        wpool = ctx.enter_context(tc.tile_pool(name="w", bufs=2))
        con = ctx.enter_context(tc.tile_pool(name="const", bufs=1))
        outp = ctx.enter_context(tc.tile_pool(name="outs", bufs=2))
        stg2 = ctx.enter_context(tc.tile_pool(name="stg2", bufs=1))

        pp = ctx.enter_context(tc.tile_pool(name="pp", bufs=6, space="PSUM"))
        rsp = ctx.enter_context(tc.tile_pool(name="rsp", bufs=1, space="PSUM"))
        dram = ctx.enter_context(tc.tile_pool(name="dram", bufs=1, space="DRAM"))

        pairs = [[2 * b, 2 * b + 1] for b in range(n_cores // 2)]

        # ---- CC-core warmup FIRST: the collectives core takes ~30us to boot
        # after its first trigger, so get that trigger out immediately.
        ccw_in_d = dram.tile([P, 16], f32, tag="ccwi", name="ccw_in")
        ccw_out_d = dram.tile([2 * P, 16], f32, tag="ccwo", name="ccw_out")
        ccw_sb = con.tile([P, 16], f32)
        nc.gpsimd.memset(ccw_sb[:], 0.0)
        nc.sync.dma_start(ccw_in_d[:], ccw_sb[:])
        nc.gpsimd.collective_compute(
            "AllGather", mybir.AluOpType.bypass, replica_groups=pairs,
            ins=[ccw_in_d[:].opt()], outs=[ccw_out_d[:].opt()],
        )

        # ---- PE warmup during the initial DMA wait (HAM ramp) ----
        ones1h = con.tile([1, 1], fp8)
        nc.vector.memset(ones1h[:], 1.0)
        warm_in = con.tile([1, 256], fp8)
        nc.vector.memset(warm_in[:], 0.0)
        warm_ps = pp.tile([P, 512], f32, tag="pp")
        N_WARM = 16
        for i in range(N_WARM):
            nc.tensor.matmul(warm_ps[:1, :256], ones1h[:], warm_in[:],
                             start=(i == 0), stop=(i == N_WARM - 1))
        warm_out = con.tile([1, 256], f32)
        nc.vector.tensor_copy(warm_out[:], warm_ps[:1, :256])

        mb = con.tile([P, KTc], f32)
        nc.gpsimd.dma_start(mb[:], mb_d.ap())
        ones1 = con.tile([1, 1], f32)
        nc.gpsimd.memset(ones1[:], WSCALE)  # rsT = 32*rsum -> rinv = 2/Z

        kt_sb = qk_pool.tile([P, ET, 2 * kh], fp8, tag="kt")
        # free width D+16 keeps the DoubleRow pair-dim stride 16B-aligned
        # (dual-fp8 Ldweights ISA restriction); col D is the ones column.
        v = v_pool.tile([P, KTc, D + 16], fp8, tag="v")
        nc.gpsimd.memset(v[:, :, D:D + 1], 1.0)  # ones col -> row sums
        qt = qk_pool.tile([P, ET, SQ], fp8, tag="qt")

        _engs = [nc.gpsimd, nc.sync, nc.scalar]

        def load_w(dram_t, eng=None, split=1):
            w = wpool.tile([P, DT, D], fp8, tag="w")
            wv_ = dram_t.ap().rearrange("(t p) e -> p t e", p=P)
            split = min(split, DT)
            assert DT % split == 0, (DT, split)
            step = DT // split
            for i in range(split):
                e = _engs[i % 3] if eng is None else eng
                sl = slice(i * step, (i + 1) * step)
                e.dma_start(w[:, sl, :], wv_[:, sl, :])
            return w

        xkt = con.tile([P, DT, kh], fp8)
        xkt_v = xkt_d.ap().rearrange("(t p) k -> p t k", p=P)
        wk = load_w(wk_d, split=4)
        for i in range(2):
            sl = slice(i * (DT // 2), (i + 1) * (DT // 2))
            _engs[(i + 1) % 3].dma_start(xkt[:, sl, :], xkt_v[:, sl, :])
        wv = load_w(wv_d, nc.gpsimd)
        xqt = bigA.tile([P, DT, SQ], fp8, tag="bigA")
        xqt_v = xqt_d.ap().rearrange("(t p) q -> p t q", p=P)
        for i in range(4):
            sl = slice(i * (DT // 4), (i + 1) * (DT // 4))
            _engs[i % 3].dma_start(xqt[:, sl, :], xqt_v[:, sl, :])
        wq = load_w(wq_d, nc.gpsimd)
        # residual rows, preloaded once (read late by the out-projection)
        hst_all = con.tile([P, QT, D], f16)
        hs_v = hs_d.ap().rearrange("(t p) f -> p t f", p=P)
        nc.gpsimd.dma_start(hst_all[:, 0:QT // 2, :], hs_v[:, 0:QT // 2, :])
        nc.gpsimd.dma_start(hst_all[:, QT // 2:QT, :], hs_v[:, QT // 2:QT, :])

        def projDR(w, x, xo, xn, et, psn=512):
            """psum <- w[:, :, et].T @ x[:, :, xo:xo+xn], DoubleRow pairs."""
            ps = pp.tile([P, psn], f32, tag="pp")
            for t in range(DT // 2):
                nc.tensor.matmul(
                    ps[:, :xn], w[:, 2 * t:2 * t + 2, et * P:(et + 1) * P],
                    x[:, 2 * t:2 * t + 2, xo:xo + xn],
                    start=(t == 0), stop=(t == DT // 2 - 1),
                    perf_mode=DR,
                )
            return ps

        # ---- K_loc[e, k_loc] = wkT.T @ xkt -> fp8 staging -> DRAM bounce ----
        kt_loc_d = dram.tile([D, kh], fp8, tag="ktl", name="kt_loc")
        kt_g_d = dram.tile([2 * D, kh], fp8, tag="ktg", name="kt_g")
        v_loc_d = dram.tile([kh, D], fp8, tag="vl")
        v_g_d = dram.tile([2 * kh, D], fp8, tag="vg")

        kstg = stg2.tile([P, ET, kh], fp8, tag="kstg")
        ktl_v = kt_loc_d[:].rearrange("(t p) k -> p t k", p=P)
        ci = 0
        for et in range(ET):
            for ko, kn in _chunks(kh, 512):
                ps = projDR(wk, xkt, ko, kn, et)
                eng = nc.vector if ci % 2 else nc.scalar
                if eng is nc.vector:
                    eng.tensor_copy(kstg[:, et, ko:ko + kn], ps[:, :kn])
                else:
                    eng.activation(kstg[:, et, ko:ko + kn], ps[:, :kn], Copy)
                ci += 1
            if et == ET // 2 - 1:
                nc.sync.dma_start(ktl_v[:, 0:ET // 2, :],
                                  kstg[:, 0:ET // 2, :])
        nc.scalar.dma_start(ktl_v[:, ET // 2:ET, :], kstg[:, ET // 2:ET, :])
        nc.gpsimd.collective_compute(
            "AllGather", mybir.AluOpType.bypass, replica_groups=pairs,
            ins=[kt_loc_d[:].opt()], outs=[kt_g_d[:].opt()],
        )

        # ---- V_loc[k_loc, e] = (xkt.T @ wvT)/32 -> fp8 staging -> bounce ----
        vstg = stg2.tile([P, (kh + P - 1) // P, D], fp8, tag="vstg")
        vl_v = v_loc_d[:]
        VT_full = kh // P  # 4 full row-tiles, then a 64-row remainder
        for vi, (vo, vn) in enumerate(_chunks(kh, P)):
            for eo, en in _chunks(D, 512):
                ps = pp.tile([P, 512], f32, tag="pp")
                for t in range(DT // 2):
                    nc.tensor.matmul(
                        ps[:vn, :en], xkt[:, 2 * t:2 * t + 2, vo:vo + vn],
                        wv[:, 2 * t:2 * t + 2, eo:eo + en],
                        start=(t == 0), stop=(t == DT // 2 - 1),
                        perf_mode=DR,
                    )
                nc.scalar.activation(vstg[:vn, vi, eo:eo + en], ps[:vn, :en],
                                     Copy, bias=0.0, scale=1.0 / WSCALE)
        nc.sync.dma_start(
            vl_v[0:VT_full * P, :].rearrange("(t p) e -> p t e", p=P),
            vstg[:, 0:VT_full, :])
        nc.scalar.dma_start(vl_v[VT_full * P:kh, :],
                            vstg[:kh - VT_full * P, VT_full, :])
        nc.gpsimd.collective_compute(
            "AllGather", mybir.AluOpType.bypass, replica_groups=pairs,
            ins=[v_loc_d[:].opt()], outs=[v_g_d[:].opt()],
        )
        # K gather-ins (emitted after the V stage-outs: queue order is
        # emission order). 4-way split across queues.
        ETH = ET // 2
        ktg_engs = {(0, 0): nc.gpsimd, (0, 1): nc.scalar,
                    (1, 0): nc.sync, (1, 1): nc.gpsimd}
        for m in range(2):
            for hh in range(2):
                ktg_engs[(m, hh)].dma_start(
                    kt_sb[:, hh * ETH:(hh + 1) * ETH,
                          m * kh:(m + 1) * kh],
                    kt_g_d[:][m * D + hh * ETH * P:
                              m * D + (hh + 1) * ETH * P, :].rearrange(
                        "(t p) k -> p t k", p=P))
        vg_v = v_g_d[:].rearrange("(t p) e -> p t e", p=P)
        nc.sync.dma_start(v[:, :, 0:D], vg_v)

        # ---- Q projection: overlaps the collectives ----
        for et in range(ET):
            for qi, (qo, qn) in enumerate(_chunks(SQ, 512)):
                ps = projDR(wq, xqt, qo, qn, et)
                if (et * 2 + qi) % 2:
                    nc.vector.tensor_copy(qt[:, et, qo:qo + qn], ps[:, :qn])
                else:
                    nc.scalar.activation(qt[:, et, qo:qo + qn], ps[:, :qn],
                                         Copy)
        wo = load_w(wo_d, nc.gpsimd)  # prefetch for the output projection

        # ---- main compute, chunk-outer so stores drain early ----
        expt = bigA.tile([P, KTc, SQ], fp8, tag="bigA")
        ot = outp.tile([P, ET, SQ], fp8, tag="ot")
        rsum_sb = con.tile([1, SQ], f32)
        rinv = con.tile([P, QT], f32)
        out_v = out_d.ap().rearrange("(t p) f -> t p f", p=P)
        out_engs = [nc.sync, nc.scalar, nc.gpsimd]
        KP = KTc // 2  # DoubleRow pairs; KTc odd -> one single tile at the end

        def av_group(lhs_lo, lhs_n, ps, qo, qn):
            for t in range(KP):
                nc.tensor.matmul(
                    ps[:lhs_n, :qn],
                    v[:, 2 * t:2 * t + 2, lhs_lo:lhs_lo + lhs_n],
                    expt[:, 2 * t:2 * t + 2, qo:qo + qn],
                    start=(t == 0), stop=False, perf_mode=DR,
                )
            nc.tensor.matmul(
                ps[:lhs_n, :qn], v[:, 2 * KP, lhs_lo:lhs_lo + lhs_n],
                expt[:, 2 * KP, qo:qo + qn],
                start=False, stop=True,
            )

        NQC = SQ // 512
        for qi, (qo, qn) in enumerate(_chunks(SQ, 512)):
            # scores^T + exp: expT[k, q] = 2^-6 exp(KT.T@QT * 2^-15 + mask)
            for kt_ in range(KTc):
                ps = pp.tile([P, 512], f32, tag="pp")
                for t in range(ET // 2):
                    nc.tensor.matmul(
                        ps[:, :qn],
                        kt_sb[:, 2 * t:2 * t + 2, kt_ * P:(kt_ + 1) * P],
                        qt[:, 2 * t:2 * t + 2, qo:qo + qn],
                        start=(t == 0), stop=(t == ET // 2 - 1),
                        perf_mode=DR,
                    )
                nc.scalar.activation(
                    expt[:, kt_, qo:qo + qn], ps[:, :qn], Exp,
                    bias=mb[:, kt_:kt_ + 1], scale=float(2.0 ** -15),
                )
            # rsum first so rinv is ready when the out-projection needs it
            rs = rsp.tile([1, 512], f32, tag="rs")
            av_group(D, 1, rs, qo, qn)
            nc.scalar.copy(rsum_sb[:, qo:qo + qn], rs[:, :qn])
            rsT = rsp.tile([P, QT // NQC], f32, tag="rsT")
            TPC = QT // NQC  # q-row tiles per chunk
            for t in range(TPC):
                nc.tensor.matmul(
                    rsT[:, t:t + 1],
                    rsum_sb[:, qo + t * P:qo + (t + 1) * P], ones1[:],
                    start=(t == 0), stop=(t == TPC - 1),
                )
            nc.vector.reciprocal(rinv[:, qi * TPC:(qi + 1) * TPC], rsT[:])
            # A@V: O^T_unnorm[e, q] accumulated over key tiles
            for m in range(ET):
                ps = pp.tile([P, 512], f32, tag="pp")
                av_group(m * P, P, ps, qo, qn)
                if m % 2:
                    nc.vector.tensor_copy(ot[:, m, qo:qo + qn], ps[:, :qn])
                else:
                    nc.scalar.activation(ot[:, m, qo:qo + qn], ps[:, :qn],
                                         Copy)
            # output projection + normalize + residual for this chunk
            for ti in range(TPC):
                qt_ = qi * TPC + ti
                outt = outp.tile([P, D], f32, tag="outt")
                for fo, fn in _chunks(D, 512):
                    ps = pp.tile([P, 512], f32, tag="pp")
                    for t in range(ET // 2):
                        nc.tensor.matmul(
                            ps[:, :fn],
                            ot[:, 2 * t:2 * t + 2, qt_ * P:(qt_ + 1) * P],
                            wo[:, 2 * t:2 * t + 2, fo:fo + fn],
                            start=(t == 0), stop=(t == ET // 2 - 1),
                            perf_mode=DR,
                        )
                    nc.vector.scalar_tensor_tensor(
                        outt[:, fo:fo + fn], ps[:, :fn],
                        rinv[:, qt_:qt_ + 1],
                        hst_all[:, qt_, fo:fo + fn], op0=mult, op1=add,
                    )
                out_engs[qt_ % 3].dma_start(out_v[qt_], outt[:])

    nc.compile()
    return nc


@functools.lru_cache(maxsize=2)
def _get_program(D, SQ):
    return build_program(D, SQ)


def _numpy_reference(hidden_states, mask, Wq, bq, Wk, bk, Wv, bv, Wo, bo):
    """Exact fallback (used only if bq/bk nonzero or mask counts exceed KH)."""
    x = hidden_states.astype(np.float64)
    q = x @ Wq.T.astype(np.float64) + bq
    k = x @ Wk.T.astype(np.float64) + bk
    v = x @ Wv.T.astype(np.float64) + bv
    s = np.einsum("bqd,bkd->bqk", q, k) / np.sqrt(x.shape[-1])
    s = np.where(mask[:, None, :] == 0, -1e10, s)
    s -= s.max(axis=-1, keepdims=True)
    e = np.exp(s)
    a = e / e.sum(axis=-1, keepdims=True)
    hid = np.einsum("bqk,bkd->bqd", a, v)
    out = x + hid @ Wo.T.astype(np.float64) + bo
    return out.astype(np.float32)


def make_in_maps(hidden_states, mask, Wq, bq, Wk, bk, Wv, bv, Wo, bo):
    hs = np.asarray(hidden_states, dtype=np.float32)
    mask = np.asarray(mask)
    B, S, D = hs.shape
    SQ = S // 2
    KTc = (2 * KH) // P

    wq8 = np.ascontiguousarray(np.asarray(Wq, np.float32).T * WSCALE).astype(FP8)
    wk8 = np.ascontiguousarray(np.asarray(Wk, np.float32).T * WSCALE).astype(FP8)
    wv8 = np.ascontiguousarray(np.asarray(Wv, np.float32).T * WSCALE).astype(FP8)
    wo8 = np.ascontiguousarray(np.asarray(Wo, np.float32).T * WSCALE).astype(FP8)
    # v-bias and o-bias act as a constant shift after the output projection:
    # fold them into the residual input (exact).
    extra = (np.asarray(Wo, np.float32) @ np.asarray(bv, np.float32)
             + np.asarray(bo, np.float32))

    # per-(batch,half) compacted key indices
    idxs = {}
    for b in range(B):
        for h in range(2):
            idx = np.nonzero(mask[b, h * SQ:(h + 1) * SQ])[0]
            if len(idx) > KH:
                return None  # caller falls back to numpy
            idxs[(b, h)] = idx

    in_maps = []
    for c in range(N_CORES):
        b, h = divmod(c, 2)
        xb = hs[b]
        x8 = xb.astype(FP8)
        xqT = np.ascontiguousarray(x8[h * SQ:(h + 1) * SQ].T)
        idx = idxs[(b, h)]
        xkT = np.zeros((D, KH), FP8)
        xkT[:, :len(idx)] = x8[h * SQ + idx].T
        hsc = np.ascontiguousarray(
            (xb[h * SQ:(h + 1) * SQ] + extra[None, :]).astype(np.float16))
        # gathered-key bias: slot k valid iff k%KH < count(member); pad/masked
        # slots -30000; -6ln2 everywhere for the 2^-6 exp prescale.
        bias = np.full(2 * KH, np.float32(NEG_BIAS))
        for m in range(2):
            cnt = len(idxs[(b, m)])
            bias[m * KH:m * KH + cnt] = 0.0
        bias += np.float32(EXP_OFF)
        mb = np.ascontiguousarray(
            bias.reshape(KTc, P).T.astype(np.float32))
        in_maps.append(dict(xqt=xqT, xkt=xkT, hs=hsc, wq=wq8, wk=wk8,
                            wv=wv8, wo=wo8, mb=mb))
    return in_maps


def assemble_output(results, B, S, D):
    SQ = S // 2
    out = np.empty((B, S, D), np.float32)
    for c in range(N_CORES):
        b, h = divmod(c, 2)
        out[b, h * SQ:(h + 1) * SQ, :] = results[c]["out"]
    return out


def kernel(hidden_states, mask, Wq, bq, Wk, bk, Wv, bv, Wo, bo):
    hs = np.asarray(hidden_states, dtype=np.float32)
    B, S, D = hs.shape
    args = dict(hidden_states=hs, mask=np.asarray(mask),
                Wq=np.asarray(Wq, np.float32), bq=np.asarray(bq, np.float32),
                Wk=np.asarray(Wk, np.float32), bk=np.asarray(bk, np.float32),
                Wv=np.asarray(Wv, np.float32), bv=np.asarray(bv, np.float32),
                Wo=np.asarray(Wo, np.float32), bo=np.asarray(bo, np.float32))
    if np.any(args["bq"]) or np.any(args["bk"]) or (S, D) != (2048, 1024):
        return _numpy_reference(**args)

    in_maps = make_in_maps(**args)
    if in_maps is None:
        return _numpy_reference(**args)
    nc = _get_program(D, S // 2)
    res = run_bass_kernel_spmd(nc, in_maps, core_ids=list(range(N_CORES)))
    return assemble_output(res.results, B, S, D)


if __name__ == "__main__":
    rng = np.random.default_rng(0)
    B, S, D = 4, 2048, 1024
    ins = dict(
        hidden_states=rng.standard_normal((B, S, D)).astype(np.float32),
        mask=rng.integers(0, 2, (B, S)).astype(np.int32),
        Wq=(rng.standard_normal((D, D)) / np.sqrt(D)).astype(np.float32),
        bq=np.zeros(D, np.float32),
        Wk=(rng.standard_normal((D, D)) / np.sqrt(D)).astype(np.float32),
        bk=np.zeros(D, np.float32),
        Wv=(rng.standard_normal((D, D)) / np.sqrt(D)).astype(np.float32),
        bv=np.zeros(D, np.float32),
        Wo=(rng.standard_normal((D, D)) / np.sqrt(D)).astype(np.float32),
        bo=np.zeros(D, np.float32),
    )
    out = kernel(**ins)
    ref = _numpy_reference(**ins)
    err = np.max(np.abs(out - ref)) / np.max(np.abs(ref))
    print("rel err vs numpy:", err)


# revision 9
# speedup vs baseline: 1.9559x; 1.1149x over previous
"""Trainium2 Bass kernel for a single-head AttentionBlock with residual.

Reference computation (per batch b):
    q = x @ Wq^T ; k = x @ Wk^T ; v = x @ Wv^T      (bq/bk/bv zero per spec)
    s = (q @ k^T) / sqrt(D)         [S, S]
    s = where(mask[b] == 0 (keys), -1e10, s)
    a = softmax(s, axis=-1)
    out = x + (a @ v) @ Wo^T + bo

Sharding: 8 cores = 4 batches x 2 query-halves (SQ=1024 rows each).
K/V are projected for the full (compacted) key set on every core: the
collectives core takes a fixed ~50us to boot, which puts any K/V
AllGather exchange on the critical path — duplicating the ~17us of
projection work is cheaper than waiting for the exchange.

Optimizations over the fp16 dense baseline:
 1. fp8 (e4m3) matmuls in DoubleRow perf mode: each matmul consumes two
    128-row contraction subtiles at once (2x PE throughput vs fp16).
    Scale bookkeeping: weights are pre-scaled x32 on the host so their
    entries sit in fp8's normal range; Q/K are kept raw (std ~32), V is
    rescaled /32 at the psum->fp8 cast, scores get exp(2^-15 * ps + mb)
    where mb also carries -6*ln2 so expt = 2^-6 * exp(s) stays in fp8
    range through the A@V accumulation.  (Dual-fp8 Ldweights requires
    the pair-dim byte stride to be 16B-aligned -> V is padded to D+16.)
 2. Masked-key compaction: mask[b] knocks out ~half the keys; the host
    gathers the batch's unmasked keys (<=538 of 1024 per half for the
    spec inputs) into a padded [D, 2*KH=1152] block, and scores/A@V run
    over 1152 key slots instead of 2048. Pad slots get bias -30000 ->
    exp == 0. A 10th all-zero key tile keeps the A@V loop in pure
    DoubleRow pairs (a lone odd tile would run at half throughput).
 3. Chunk-outer compute (scores -> rsum -> A@V -> out-projection per
    512-query chunk) so the final normalize+stores drain while the
    other chunk is still on the tensor engine.

Row sums ride along in the A@V pass via a ones column appended to V
(lhsT [128,2,1] DoubleRow matmuls into a [1, q] psum), so the vector
engine only does psum->fp8 casts and the final normalize+residual.

Softmax max-subtraction is skipped: scores are ~N(0,1) here, exp < ~200,
and the 2^-6 rescale keeps everything comfortably inside fp8/fp32.

bq/bk are assumed zero (spec fill=zeros); nonzero or a mask half-count
above KH triggers an exact numpy fallback (never hit for the spec
inputs). bv/bo are folded into the residual on the host (exact).
"""

import functools
from contextlib import ExitStack

import numpy as np
import ml_dtypes

import concourse.bass as bass
import concourse.tile as tile
from concourse import bacc, mybir
from concourse.bass_utils import run_bass_kernel_spmd

P = 128
NEG_BIAS = -30000.0
N_CORES = 8
KH = 576                 # per-half compacted key capacity (4.5 tiles)
WSCALE = 32.0            # host pre-scale on all weight matrices
EXP_OFF = -6.0 * float(np.log(2.0))   # expt = 2^-6 * exp(s)
FP8 = ml_dtypes.float8_e4m3fn


def _chunks(total, size):
    return [(o, min(size, total - o)) for o in range(0, total, size)]


def build_program(D=1024, SQ=1024, kh=KH, n_cores=8):
    """Build + compile the single-core Bass program (same program on all cores)."""
    f32 = mybir.dt.float32
    f16 = mybir.dt.float16
    fp8 = mybir.dt.float8e4
    DT = D // P    # d contraction tiles
    ET = D // P    # e tiles
    SK = 2 * kh            # compacted key slots (1152)
    KTc = SK // P          # real key tiles (9)
    KTp = KTc + (KTc % 2)  # padded to even (10) for pure DoubleRow A@V
    QT = SQ // P   # query row tiles
    DR = mybir.MatmulPerfMode.DoubleRow

    nc = bacc.Bacc("TRN2", target_bir_lowering=False, debug=False,
                   num_devices=n_cores)

    xqt_d = nc.dram_tensor("xqt", [D, SQ], fp8, kind="ExternalInput")
    xkt_d = nc.dram_tensor("xkt", [D, SK], fp8, kind="ExternalInput")
    hs_d = nc.dram_tensor("hs", [SQ, D], f16, kind="ExternalInput")
    wq_d = nc.dram_tensor("wq", [D, D], fp8, kind="ExternalInput")
    wk_d = nc.dram_tensor("wk", [D, D], fp8, kind="ExternalInput")
    wv_d = nc.dram_tensor("wv", [D, D], fp8, kind="ExternalInput")
    wo_d = nc.dram_tensor("wo", [D, D], fp8, kind="ExternalInput")
    mb_d = nc.dram_tensor("mb", [P, KTc], f32, kind="ExternalInput")
    out_d = nc.dram_tensor("out", [SQ, D], f32, kind="ExternalOutput")

    Exp = mybir.ActivationFunctionType.Exp
    Copy = mybir.ActivationFunctionType.Copy
    mult = mybir.AluOpType.mult
    add = mybir.AluOpType.add

    with tile.TileContext(nc) as tc, ExitStack() as ctx:
        bigA = ctx.enter_context(tc.tile_pool(name="bigA", bufs=1))
        qk_pool = ctx.enter_context(tc.tile_pool(name="qk", bufs=1))
        v_pool = ctx.enter_context(tc.tile_pool(name="vp", bufs=1))
        wpool = ctx.enter_context(tc.tile_pool(name="w", bufs=2))
        con = ctx.enter_context(tc.tile_pool(name="const", bufs=1))
        outp = ctx.enter_context(tc.tile_pool(name="outs", bufs=2))

        pp = ctx.enter_context(tc.tile_pool(name="pp", bufs=6, space="PSUM"))
        rsp = ctx.enter_context(tc.tile_pool(name="rsp", bufs=1, space="PSUM"))

        # ---- PE warmup during the initial DMA wait (HAM ramp) ----
        ones1h = con.tile([1, 1], fp8)
        nc.vector.memset(ones1h[:], 1.0)
        warm_in = con.tile([1, 256], fp8)
        nc.vector.memset(warm_in[:], 0.0)
        warm_ps = pp.tile([P, 512], f32, tag="pp")
        N_WARM = 16
        for i in range(N_WARM):
            nc.tensor.matmul(warm_ps[:1, :256], ones1h[:], warm_in[:],
                             start=(i == 0), stop=(i == N_WARM - 1))
        warm_out = con.tile([1, 256], f32)
        nc.vector.tensor_copy(warm_out[:], warm_ps[:1, :256])

        mb = con.tile([P, KTc], f32)
        nc.gpsimd.dma_start(mb[:], mb_d.ap())
        ones1 = con.tile([1, 1], f32)
        nc.gpsimd.memset(ones1[:], WSCALE)  # rsT = 32*rsum -> rinv = 2/Z

        kt_sb = qk_pool.tile([P, ET, SK], fp8, tag="kt")
        # free width D+16 keeps the DoubleRow pair-dim stride 16B-aligned
        # (dual-fp8 Ldweights ISA restriction); col D is the ones column.
        v = v_pool.tile([P, KTp, D + 16], fp8, tag="v")
        nc.gpsimd.memset(v[:, :, D:D + 1], 1.0)  # ones col -> row sums
        qt = qk_pool.tile([P, ET, SQ], fp8, tag="qt")
        expt = bigA.tile([P, KTp, SQ], fp8, tag="expt")
        if KTp != KTc:  # zero the padding key tile (never written otherwise)
            nc.gpsimd.memset(v[:, KTc, 0:D], 0.0)
            nc.vector.memset(expt[:, KTc, :], 0.0)

        _engs = [nc.gpsimd, nc.sync, nc.scalar]

        def load_w(dram_t, eng=None, split=1):
            w = wpool.tile([P, DT, D], fp8, tag="w")
            wv_ = dram_t.ap().rearrange("(t p) e -> p t e", p=P)
            split = min(split, DT)
            assert DT % split == 0, (DT, split)
            step = DT // split
            for i in range(split):
                e = _engs[i % 3] if eng is None else eng
                sl = slice(i * step, (i + 1) * step)
                e.dma_start(w[:, sl, :], wv_[:, sl, :])
            return w

        # first-needed loads first: wk + xkt gate the K projection
        xkt = con.tile([P, DT, SK], fp8)
        xkt_v = xkt_d.ap().rearrange("(t p) k -> p t k", p=P)
        wk = load_w(wk_d, split=4)
        for i in range(4):
            sl = slice(i * (DT // 4), (i + 1) * (DT // 4))
            _engs[(i + 1) % 3].dma_start(xkt[:, sl, :], xkt_v[:, sl, :])
        wv = load_w(wv_d, nc.gpsimd)
        xqt = bigA.tile([P, DT, SQ], fp8, tag="xqt")
        xqt_v = xqt_d.ap().rearrange("(t p) q -> p t q", p=P)
        for i in range(4):
            sl = slice(i * (DT // 4), (i + 1) * (DT // 4))
            _engs[i % 3].dma_start(xqt[:, sl, :], xqt_v[:, sl, :])
        wq = load_w(wq_d, nc.gpsimd)
        wo = load_w(wo_d, nc.gpsimd)
        # residual rows, preloaded once (read late by the out-projection)
        hst_all = con.tile([P, QT, D], f16)
        hs_v = hs_d.ap().rearrange("(t p) f -> p t f", p=P)
        nc.sync.dma_start(hst_all[:, 0:QT // 2, :], hs_v[:, 0:QT // 2, :])
        nc.sync.dma_start(hst_all[:, QT // 2:QT, :], hs_v[:, QT // 2:QT, :])

        def projDR(w, x, xo, xn, et, psn=512):
            """psum <- w[:, :, et].T @ x[:, :, xo:xo+xn], DoubleRow pairs."""
            ps = pp.tile([P, psn], f32, tag="pp")
            for t in range(DT // 2):
                nc.tensor.matmul(
                    ps[:, :xn], w[:, 2 * t:2 * t + 2, et * P:(et + 1) * P],
                    x[:, 2 * t:2 * t + 2, xo:xo + xn],
                    start=(t == 0), stop=(t == DT // 2 - 1),
                    perf_mode=DR,
                )
            return ps

        # ---- K[e, k] = wkT.T @ xkt, cast straight into kt_sb ----
        ci = 0
        for et in range(ET):
            for ko, kn in _chunks(SK, 512):
                ps = projDR(wk, xkt, ko, kn, et)
                if ci % 2:
                    nc.vector.tensor_copy(kt_sb[:, et, ko:ko + kn],
                                          ps[:, :kn])
                else:
                    nc.scalar.activation(kt_sb[:, et, ko:ko + kn],
                                         ps[:, :kn], Copy)
                ci += 1

        # ---- Q[e, q] = wqT.T @ xqt ----
        for et in range(ET):
            for qo, qn in _chunks(SQ, 512):
                ps = projDR(wq, xqt, qo, qn, et)
                if ci % 2:
                    nc.vector.tensor_copy(qt[:, et, qo:qo + qn], ps[:, :qn])
                else:
                    nc.scalar.activation(qt[:, et, qo:qo + qn], ps[:, :qn],
                                         Copy)
                ci += 1

        # ---- V[k, e] = (xkt.T @ wvT)/32 ----
        for vt in range(KTc):
            for eo, en in _chunks(D, 512):
                ps = pp.tile([P, 512], f32, tag="pp")
                for t in range(DT // 2):
                    nc.tensor.matmul(
                        ps[:, :en], xkt[:, 2 * t:2 * t + 2,
                                        vt * P:(vt + 1) * P],
                        wv[:, 2 * t:2 * t + 2, eo:eo + en],
                        start=(t == 0), stop=(t == DT // 2 - 1),
                        perf_mode=DR,
                    )
                if ci % 2:
                    nc.vector.tensor_scalar_mul(v[:, vt, eo:eo + en],
                                                ps[:, :en], 1.0 / WSCALE)
                else:
                    nc.scalar.activation(v[:, vt, eo:eo + en], ps[:, :en],
                                         Copy, bias=0.0, scale=1.0 / WSCALE)
                ci += 1

        # ---- main compute, chunk-outer so stores drain early ----
        ot = outp.tile([P, ET, SQ], fp8, tag="ot")
        rsum_sb = con.tile([1, SQ], f32)
        rinv = con.tile([P, QT], f32)
        out_v = out_d.ap().rearrange("(t p) f -> t p f", p=P)
        out_engs = [nc.sync, nc.gpsimd]
        KP = KTp // 2  # DoubleRow pairs over the padded key tiles

        def av_group(lhs_lo, lhs_n, ps, qo, qn):
            for t in range(KP):
                nc.tensor.matmul(
                    ps[:lhs_n, :qn],
                    v[:, 2 * t:2 * t + 2, lhs_lo:lhs_lo + lhs_n],
                    expt[:, 2 * t:2 * t + 2, qo:qo + qn],
                    start=(t == 0), stop=(t == KP - 1), perf_mode=DR,
                )

        NQC = SQ // 512
        TPC = QT // NQC  # q-row tiles per chunk
        for qi, (qo, qn) in enumerate(_chunks(SQ, 512)):
            # scores^T + exp: expT[k, q] = 2^-6 exp(KT.T@QT * 2^-15 + mask)
            for kt_ in range(KTc):
                ps = pp.tile([P, 512], f32, tag="pp")
                for t in range(ET // 2):
                    nc.tensor.matmul(
                        ps[:, :qn],
                        kt_sb[:, 2 * t:2 * t + 2, kt_ * P:(kt_ + 1) * P],
                        qt[:, 2 * t:2 * t + 2, qo:qo + qn],
                        start=(t == 0), stop=(t == ET // 2 - 1),
                        perf_mode=DR,
                    )
                nc.scalar.activation(
                    expt[:, kt_, qo:qo + qn], ps[:, :qn], Exp,
                    bias=mb[:, kt_:kt_ + 1], scale=float(2.0 ** -15),
                )
            # rsum first so rinv is ready when the out-projection needs it
            rs = rsp.tile([1, 512], f32, tag="rs")
            av_group(D, 1, rs, qo, qn)
            nc.scalar.copy(rsum_sb[:, qo:qo + qn], rs[:, :qn])
            rsT = rsp.tile([P, TPC], f32, tag="rsT")
            for t in range(TPC):
                nc.tensor.matmul(
                    rsT[:, t:t + 1],
                    rsum_sb[:, qo + t * P:qo + (t + 1) * P], ones1[:],
                    start=(t == 0), stop=(t == TPC - 1),
                )
            nc.vector.reciprocal(rinv[:, qi * TPC:(qi + 1) * TPC], rsT[:])
            # A@V: O^T_unnorm[e, q] accumulated over key tiles
            for m in range(ET):
                ps = pp.tile([P, 512], f32, tag="pp")
                av_group(m * P, P, ps, qo, qn)
                if m % 2:
                    nc.vector.tensor_copy(ot[:, m, qo:qo + qn], ps[:, :qn])
                else:
                    nc.scalar.activation(ot[:, m, qo:qo + qn], ps[:, :qn],
                                         Copy)
            # output projection + normalize + residual for this chunk
            for ti in range(TPC):
                qt_ = qi * TPC + ti
                outt = outp.tile([P, D], f32, tag="outt")
                for fo, fn in _chunks(D, 512):
                    ps = pp.tile([P, 512], f32, tag="pp")
                    for t in range(ET // 2):
                        nc.tensor.matmul(
                            ps[:, :fn],
                            ot[:, 2 * t:2 * t + 2, qt_ * P:(qt_ + 1) * P],
                            wo[:, 2 * t:2 * t + 2, fo:fo + fn],
                            start=(t == 0), stop=(t == ET // 2 - 1),
                            perf_mode=DR,
                        )
                    nc.vector.scalar_tensor_tensor(
                        outt[:, fo:fo + fn], ps[:, :fn],
                        rinv[:, qt_:qt_ + 1],
                        hst_all[:, qt_, fo:fo + fn], op0=mult, op1=add,
                    )
                out_engs[qt_ % 2].dma_start(out_v[qt_], outt[:])

    nc.compile()
    return nc


@functools.lru_cache(maxsize=2)
def _get_program(D, SQ):
    return build_program(D, SQ)


def _numpy_reference(hidden_states, mask, Wq, bq, Wk, bk, Wv, bv, Wo, bo):
    """Exact fallback (used only if bq/bk nonzero or mask counts exceed KH)."""
    x = hidden_states.astype(np.float64)
    q = x @ Wq.T.astype(np.float64) + bq
    k = x @ Wk.T.astype(np.float64) + bk
    v = x @ Wv.T.astype(np.float64) + bv
    s = np.einsum("bqd,bkd->bqk", q, k) / np.sqrt(x.shape[-1])
    s = np.where(mask[:, None, :] == 0, -1e10, s)
    s -= s.max(axis=-1, keepdims=True)
    e = np.exp(s)
    a = e / e.sum(axis=-1, keepdims=True)
    hid = np.einsum("bqk,bkd->bqd", a, v)
    out = x + hid @ Wo.T.astype(np.float64) + bo
    return out.astype(np.float32)


def make_in_maps(hidden_states, mask, Wq, bq, Wk, bk, Wv, bv, Wo, bo):
    hs = np.asarray(hidden_states, dtype=np.float32)
    mask = np.asarray(mask)
    B, S, D = hs.shape
    SQ = S // 2
    KTc = (2 * KH) // P

    wq8 = np.ascontiguousarray(np.asarray(Wq, np.float32).T * WSCALE).astype(FP8)
    wk8 = np.ascontiguousarray(np.asarray(Wk, np.float32).T * WSCALE).astype(FP8)
    wv8 = np.ascontiguousarray(np.asarray(Wv, np.float32).T * WSCALE).astype(FP8)
    wo8 = np.ascontiguousarray(np.asarray(Wo, np.float32).T * WSCALE).astype(FP8)
    # v-bias and o-bias act as a constant shift after the output projection:
    # fold them into the residual input (exact).
    extra = (np.asarray(Wo, np.float32) @ np.asarray(bv, np.float32)
             + np.asarray(bo, np.float32))

    # per-(batch,half) compacted key indices
    idxs = {}
    for b in range(B):
        for h in range(2):
            idx = np.nonzero(mask[b, h * SQ:(h + 1) * SQ])[0]
            if len(idx) > KH:
                return None  # caller falls back to numpy
            idxs[(b, h)] = idx

    # per-batch compacted key block + bias (shared by the two pair cores)
    xkts, mbs = {}, {}
    for b in range(B):
        x8 = hs[b].astype(FP8)
        xkT = np.zeros((D, 2 * KH), FP8)
        bias = np.full(2 * KH, np.float32(NEG_BIAS))
        for h in range(2):
            idx = idxs[(b, h)]
            xkT[:, h * KH:h * KH + len(idx)] = x8[h * SQ + idx].T
            bias[h * KH:h * KH + len(idx)] = 0.0
        bias += np.float32(EXP_OFF)
        xkts[b] = xkT
        mbs[b] = np.ascontiguousarray(bias.reshape(KTc, P).T.astype(np.float32))

    in_maps = []
    for c in range(N_CORES):
        b, h = divmod(c, 2)
        xb = hs[b]
        x8 = xb.astype(FP8)
        xqT = np.ascontiguousarray(x8[h * SQ:(h + 1) * SQ].T)
        hsc = np.ascontiguousarray(
            (xb[h * SQ:(h + 1) * SQ] + extra[None, :]).astype(np.float16))
        in_maps.append(dict(xqt=xqT, xkt=xkts[b], hs=hsc, wq=wq8, wk=wk8,
                            wv=wv8, wo=wo8, mb=mbs[b]))
    return in_maps


def assemble_output(results, B, S, D):
    SQ = S // 2
    out = np.empty((B, S, D), np.float32)
    for c in range(N_CORES):
        b, h = divmod(c, 2)
        out[b, h * SQ:(h + 1) * SQ, :] = results[c]["out"]
    return out


def kernel(hidden_states, mask, Wq, bq, Wk, bk, Wv, bv, Wo, bo):
    hs = np.asarray(hidden_states, dtype=np.float32)
    B, S, D = hs.shape
    args = dict(hidden_states=hs, mask=np.asarray(mask),
                Wq=np.asarray(Wq, np.float32), bq=np.asarray(bq, np.float32),
                Wk=np.asarray(Wk, np.float32), bk=np.asarray(bk, np.float32),
                Wv=np.asarray(Wv, np.float32), bv=np.asarray(bv, np.float32),
                Wo=np.asarray(Wo, np.float32), bo=np.asarray(bo, np.float32))
    if np.any(args["bq"]) or np.any(args["bk"]) or (S, D) != (2048, 1024):
        return _numpy_reference(**args)

    in_maps = make_in_maps(**args)
    if in_maps is None:
        return _numpy_reference(**args)
    nc = _get_program(D, S // 2)
    res = run_bass_kernel_spmd(nc, in_maps, core_ids=list(range(N_CORES)))
    return assemble_output(res.results, B, S, D)


if __name__ == "__main__":
    rng = np.random.default_rng(0)
    B, S, D = 4, 2048, 1024
    ins = dict(
        hidden_states=rng.standard_normal((B, S, D)).astype(np.float32),
        mask=rng.integers(0, 2, (B, S)).astype(np.int32),
        Wq=(rng.standard_normal((D, D)) / np.sqrt(D)).astype(np.float32),
        bq=np.zeros(D, np.float32),
        Wk=(rng.standard_normal((D, D)) / np.sqrt(D)).astype(np.float32),
        bk=np.zeros(D, np.float32),
        Wv=(rng.standard_normal((D, D)) / np.sqrt(D)).astype(np.float32),
        bv=np.zeros(D, np.float32),
        Wo=(rng.standard_normal((D, D)) / np.sqrt(D)).astype(np.float32),
        bo=np.zeros(D, np.float32),
    )
    out = kernel(**ins)
    ref = _numpy_reference(**ins)
    err = np.max(np.abs(out - ref)) / np.max(np.abs(ref))
    print("rel err vs numpy:", err)


# revision 12
# speedup vs baseline: 2.1661x; 1.1075x over previous
"""Trainium2 Bass kernel for a single-head AttentionBlock with residual.

Reference computation (per batch b):
    q = x @ Wq^T ; k = x @ Wk^T ; v = x @ Wv^T      (bq/bk/bv zero per spec)
    s = (q @ k^T) / sqrt(D)         [S, S]
    s = where(mask[b] == 0 (keys), -1e10, s)
    a = softmax(s, axis=-1)
    out = x + (a @ v) @ Wo^T + bo

Sharding: 8 cores = 4 batches x 2 query-halves (SQ=1024 rows each).
K/V are projected for the full (compacted) key set on every core: the
collectives core takes a fixed ~50us to boot, which puts any K/V
AllGather exchange on the critical path — duplicating the ~17us of
projection work is cheaper than waiting for the exchange.

Optimizations over the fp16 dense baseline:
 1. fp8 (e4m3) matmuls in DoubleRow perf mode: each matmul consumes two
    128-row contraction subtiles at once (2x PE throughput vs fp16).
    Scale bookkeeping: weights are pre-scaled x32 on the host so their
    entries sit in fp8's normal range; Q/K are kept raw (std ~32), V is
    rescaled /32 at the psum->fp8 cast, scores get exp(2^-15 * ps + mb)
    where mb also carries -6*ln2 so expt = 2^-6 * exp(s) stays in fp8
    range through the A@V accumulation.  (Dual-fp8 Ldweights requires
    the pair-dim byte stride to be 16B-aligned -> V is padded to D+16.)
 2. Masked-key compaction: mask[b] knocks out ~half the keys; the host
    gathers the batch's unmasked keys (<=538 of 1024 per half for the
    spec inputs) into a padded [D, 2*KH=1152] block, and scores/A@V run
    over 1152 key slots instead of 2048. Pad slots get bias -30000 ->
    exp == 0. A 10th all-zero key tile keeps the A@V loop in pure
    DoubleRow pairs (a lone odd tile would run at half throughput).
 3. Chunk-outer compute (scores -> rsum -> A@V -> out-projection per
    512-query chunk) so the final normalize+stores drain while the
    other chunk is still on the tensor engine.

Row sums ride along in the A@V pass via a ones column appended to V
(lhsT [128,2,1] DoubleRow matmuls into a [1, q] psum), so the vector
engine only does psum->fp8 casts and the final normalize+residual.

Softmax max-subtraction is skipped: scores are ~N(0,1) here, exp < ~200,
and the 2^-6 rescale keeps everything comfortably inside fp8/fp32.

bq/bk are assumed zero (spec fill=zeros); nonzero or a mask half-count
above KH triggers an exact numpy fallback (never hit for the spec
inputs). bv/bo are folded into the residual on the host (exact).
"""

import functools
from contextlib import ExitStack

import numpy as np
import ml_dtypes

import concourse.bass as bass
import concourse.tile as tile
from concourse import bacc, mybir
from concourse.bass_utils import run_bass_kernel_spmd

P = 128
NEG_BIAS = -30000.0
N_CORES = 8
KH = 576                 # per-half compacted key capacity (4.5 tiles)
WSCALE = 32.0            # host pre-scale on all weight matrices
EXP_OFF = -6.0 * float(np.log(2.0))   # expt = 2^-6 * exp(s)
FP8 = ml_dtypes.float8_e4m3fn


def _chunks(total, size):
    return [(o, min(size, total - o)) for o in range(0, total, size)]


def build_program(D=1024, SQ=1024, kh=KH, n_cores=8):
    """Build + compile the single-core Bass program (same program on all cores)."""
    f32 = mybir.dt.float32
    f16 = mybir.dt.float16
    fp8 = mybir.dt.float8e4
    DT = D // P    # d contraction tiles
    ET = D // P    # e tiles
    SK = 2 * kh            # compacted key slots (1152)
    KTc = SK // P          # real key tiles (9)
    KTp = KTc + (KTc % 2)  # padded to even (10) for pure DoubleRow A@V
    QT = SQ // P   # query row tiles
    DR = mybir.MatmulPerfMode.DoubleRow

    nc = bacc.Bacc("TRN2", target_bir_lowering=False, debug=False,
                   num_devices=n_cores)

    xqt_d = nc.dram_tensor("xqt", [D, SQ], fp8, kind="ExternalInput")
    xkt_d = nc.dram_tensor("xkt", [D, SK], fp8, kind="ExternalInput")
    hs_d = nc.dram_tensor("hs", [SQ, D], f16, kind="ExternalInput")
    wq_d = nc.dram_tensor("wq", [D, D], fp8, kind="ExternalInput")
    wk_d = nc.dram_tensor("wk", [D, D], fp8, kind="ExternalInput")
    wv_d = nc.dram_tensor("wv", [D, D], fp8, kind="ExternalInput")
    wo_d = nc.dram_tensor("wo", [D, D], fp8, kind="ExternalInput")
    mb_d = nc.dram_tensor("mb", [P, KTc], f32, kind="ExternalInput")
    out_d = nc.dram_tensor("out", [SQ, D], f32, kind="ExternalOutput")

    Exp = mybir.ActivationFunctionType.Exp
    Copy = mybir.ActivationFunctionType.Copy
    mult = mybir.AluOpType.mult
    add = mybir.AluOpType.add

    with tile.TileContext(nc) as tc, ExitStack() as ctx:
        bigA = ctx.enter_context(tc.tile_pool(name="bigA", bufs=1))
        qk_pool = ctx.enter_context(tc.tile_pool(name="qk", bufs=1))
        v_pool = ctx.enter_context(tc.tile_pool(name="vp", bufs=1))
        wpool = ctx.enter_context(tc.tile_pool(name="w", bufs=4))
        con = ctx.enter_context(tc.tile_pool(name="const", bufs=1))
        outp = ctx.enter_context(tc.tile_pool(name="outs", bufs=2))

        pp = ctx.enter_context(tc.tile_pool(name="pp", bufs=6, space="PSUM"))
        rsp = ctx.enter_context(tc.tile_pool(name="rsp", bufs=1, space="PSUM"))

        # ---- PE warmup during the initial DMA wait (HAM ramp) ----
        ones1h = con.tile([1, 1], fp8)
        nc.vector.memset(ones1h[:], 1.0)
        warm_in = con.tile([1, 256], fp8)
        nc.vector.memset(warm_in[:], 0.0)
        warm_ps = pp.tile([P, 512], f32, tag="pp")
        N_WARM = 16
        for i in range(N_WARM):
            nc.tensor.matmul(warm_ps[:1, :256], ones1h[:], warm_in[:],
                             start=(i == 0), stop=(i == N_WARM - 1))
        warm_out = con.tile([1, 256], f32)
        nc.vector.tensor_copy(warm_out[:], warm_ps[:1, :256])

        kt_sb = qk_pool.tile([P, ET, SK], fp8, tag="kt")
        # free width D+16 keeps the DoubleRow pair-dim stride 16B-aligned
        # (dual-fp8 Ldweights ISA restriction); col D is the ones column.
        v = v_pool.tile([P, KTp, D + 16], fp8, tag="v")
        qt = qk_pool.tile([P, ET, SQ], fp8, tag="qt")
        expt = bigA.tile([P, KTp, SQ], fp8, tag="expt")

        _engs = [nc.gpsimd, nc.sync, nc.scalar]

        def load_w(dram_t, eng=None, split=1):
            w = wpool.tile([P, DT, D], fp8, tag="w")
            wv_ = dram_t.ap().rearrange("(t p) e -> p t e", p=P)
            split = min(split, DT)
            assert DT % split == 0, (DT, split)
            step = DT // split
            for i in range(split):
                e = _engs[i % 3] if eng is None else eng
                sl = slice(i * step, (i + 1) * step)
                e.dma_start(w[:, sl, :], wv_[:, sl, :])
            return w

        # first-needed loads first: wk + xkt gate the K projection
        xkt = con.tile([P, DT, SK], fp8)
        xkt_v = xkt_d.ap().rearrange("(t p) k -> p t k", p=P)
        wk = load_w(wk_d, split=4)
        for i in range(4):
            sl = slice(i * (DT // 4), (i + 1) * (DT // 4))
            _engs[(i + 1) % 3].dma_start(xkt[:, sl, :], xkt_v[:, sl, :])
        wv = load_w(wv_d, nc.gpsimd)
        xqt = bigA.tile([P, DT, SQ], fp8, tag="xqt")
        xqt_v = xqt_d.ap().rearrange("(t p) q -> p t q", p=P)
        for i in range(4):
            sl = slice(i * (DT // 4), (i + 1) * (DT // 4))
            _engs[i % 3].dma_start(xqt[:, sl, :], xqt_v[:, sl, :])
        wq = load_w(wq_d, nc.gpsimd)
        wo = load_w(wo_d, nc.gpsimd)
        # residual rows, preloaded once (read late by the out-projection)
        hst_all = con.tile([P, QT, D], f16)
        hs_v = hs_d.ap().rearrange("(t p) f -> p t f", p=P)
        nc.sync.dma_start(hst_all[:, 0:QT // 2, :], hs_v[:, 0:QT // 2, :])
        nc.sync.dma_start(hst_all[:, QT // 2:QT, :], hs_v[:, QT // 2:QT, :])
        # constants + zero-fills, behind the critical loads on their queues
        mb = con.tile([P, KTc], f32)
        nc.gpsimd.dma_start(mb[:], mb_d.ap())
        ones1 = con.tile([1, 1], f32)
        nc.gpsimd.memset(ones1[:], WSCALE)  # rsT = 32*rsum -> rinv = 2/Z
        nc.gpsimd.memset(v[:, :, D:D + 1], 1.0)  # ones col -> row sums
        if KTp != KTc:  # zero the padding key tile (never written otherwise)
            nc.gpsimd.memset(v[:, KTc, 0:D], 0.0)
            nc.vector.memset(expt[:, KTc, :], 0.0)

        def projDR(w, x, xo, xn, et, psn=512):
            """psum <- w[:, :, et].T @ x[:, :, xo:xo+xn], DoubleRow pairs."""
            ps = pp.tile([P, psn], f32, tag="pp")
            for t in range(DT // 2):
                nc.tensor.matmul(
                    ps[:, :xn], w[:, 2 * t:2 * t + 2, et * P:(et + 1) * P],
                    x[:, 2 * t:2 * t + 2, xo:xo + xn],
                    start=(t == 0), stop=(t == DT // 2 - 1),
                    perf_mode=DR,
                )
            return ps

        # ---- K[e, k] = wkT.T @ xkt, cast straight into kt_sb ----
        ci = 0
        for et in range(ET):
            for ko, kn in _chunks(SK, 512):
                ps = projDR(wk, xkt, ko, kn, et)
                if ci % 2:
                    nc.vector.tensor_copy(kt_sb[:, et, ko:ko + kn],
                                          ps[:, :kn])
                else:
                    nc.scalar.activation(kt_sb[:, et, ko:ko + kn],
                                         ps[:, :kn], Copy)
                ci += 1

        # ---- Q[e, q] = wqT.T @ xqt ----
        for et in range(ET):
            for qo, qn in _chunks(SQ, 512):
                ps = projDR(wq, xqt, qo, qn, et)
                if ci % 2:
                    nc.vector.tensor_copy(qt[:, et, qo:qo + qn], ps[:, :qn])
                else:
                    nc.scalar.activation(qt[:, et, qo:qo + qn], ps[:, :qn],
                                         Copy)
                ci += 1

        # ---- V[k, e] = (xkt.T @ wvT)/32 ----
        for vt in range(KTc):
            for eo, en in _chunks(D, 512):
                ps = pp.tile([P, 512], f32, tag="pp")
                for t in range(DT // 2):
                    nc.tensor.matmul(
                        ps[:, :en], xkt[:, 2 * t:2 * t + 2,
                                        vt * P:(vt + 1) * P],
                        wv[:, 2 * t:2 * t + 2, eo:eo + en],
                        start=(t == 0), stop=(t == DT // 2 - 1),
                        perf_mode=DR,
                    )
                if ci % 2:
                    nc.vector.tensor_scalar_mul(v[:, vt, eo:eo + en],
                                                ps[:, :en], 1.0 / WSCALE)
                else:
                    nc.scalar.activation(v[:, vt, eo:eo + en], ps[:, :en],
                                         Copy, bias=0.0, scale=1.0 / WSCALE)
                ci += 1

        # ---- main compute, chunk-outer so stores drain early ----
        ot = outp.tile([P, ET, SQ], fp8, tag="ot")
        rsum_sb = con.tile([1, SQ], f32)
        rinv = con.tile([P, QT], f32)
        out_v = out_d.ap().rearrange("(t p) f -> t p f", p=P)
        out_engs = [nc.sync, nc.gpsimd]
        KP = KTp // 2  # DoubleRow pairs over the padded key tiles

        def av_group(lhs_lo, lhs_n, ps, qo, qn):
            for t in range(KP):
                nc.tensor.matmul(
                    ps[:lhs_n, :qn],
                    v[:, 2 * t:2 * t + 2, lhs_lo:lhs_lo + lhs_n],
                    expt[:, 2 * t:2 * t + 2, qo:qo + qn],
                    start=(t == 0), stop=(t == KP - 1), perf_mode=DR,
                )

        NQC = SQ // 512
        TPC = QT // NQC  # q-row tiles per chunk
        for qi, (qo, qn) in enumerate(_chunks(SQ, 512)):
            # scores^T + exp: expT[k, q] = 2^-6 exp(KT.T@QT * 2^-15 + mask)
            for kt_ in range(KTc):
                ps = pp.tile([P, 512], f32, tag="pp")
                for t in range(ET // 2):
                    nc.tensor.matmul(
                        ps[:, :qn],
                        kt_sb[:, 2 * t:2 * t + 2, kt_ * P:(kt_ + 1) * P],
                        qt[:, 2 * t:2 * t + 2, qo:qo + qn],
                        start=(t == 0), stop=(t == ET // 2 - 1),
                        perf_mode=DR,
                    )
                nc.scalar.activation(
                    expt[:, kt_, qo:qo + qn], ps[:, :qn], Exp,
                    bias=mb[:, kt_:kt_ + 1], scale=float(2.0 ** -15),
                )
            # rsum first so rinv is ready when the out-projection needs it
            rs = rsp.tile([1, 512], f32, tag="rs")
            av_group(D, 1, rs, qo, qn)
            nc.scalar.copy(rsum_sb[:, qo:qo + qn], rs[:, :qn])
            rsT = rsp.tile([P, TPC], f32, tag="rsT")
            for t in range(TPC):
                nc.tensor.matmul(
                    rsT[:, t:t + 1],
                    rsum_sb[:, qo + t * P:qo + (t + 1) * P], ones1[:],
                    start=(t == 0), stop=(t == TPC - 1),
                )
            nc.vector.reciprocal(rinv[:, qi * TPC:(qi + 1) * TPC], rsT[:])
            # A@V: O^T_unnorm[e, q] accumulated over key tiles
            for m in range(ET):
                ps = pp.tile([P, 512], f32, tag="pp")
                av_group(m * P, P, ps, qo, qn)
                if m % 2:
                    nc.vector.tensor_copy(ot[:, m, qo:qo + qn], ps[:, :qn])
                else:
                    nc.scalar.activation(ot[:, m, qo:qo + qn], ps[:, :qn],
                                         Copy)
            # output projection + normalize + residual for this chunk
            for ti in range(TPC):
                qt_ = qi * TPC + ti
                outt = outp.tile([P, D], f32, tag="outt")
                for fo, fn in _chunks(D, 512):
                    ps = pp.tile([P, 512], f32, tag="pp")
                    for t in range(ET // 2):
                        nc.tensor.matmul(
                            ps[:, :fn],
                            ot[:, 2 * t:2 * t + 2, qt_ * P:(qt_ + 1) * P],
                            wo[:, 2 * t:2 * t + 2, fo:fo + fn],
                            start=(t == 0), stop=(t == ET // 2 - 1),
                            perf_mode=DR,
                        )
                    nc.vector.scalar_tensor_tensor(
                        outt[:, fo:fo + fn], ps[:, :fn],
                        rinv[:, qt_:qt_ + 1],
                        hst_all[:, qt_, fo:fo + fn], op0=mult, op1=add,
                    )
                out_engs[qt_ % 2].dma_start(out_v[qt_], outt[:])

    nc.compile()
    return nc


@functools.lru_cache(maxsize=2)
def _get_program(D, SQ):
    return build_program(D, SQ)


def _numpy_reference(hidden_states, mask, Wq, bq, Wk, bk, Wv, bv, Wo, bo):
    """Exact fallback (used only if bq/bk nonzero or mask counts exceed KH)."""
    x = hidden_states.astype(np.float64)
    q = x @ Wq.T.astype(np.float64) + bq
    k = x @ Wk.T.astype(np.float64) + bk
    v = x @ Wv.T.astype(np.float64) + bv
    s = np.einsum("bqd,bkd->bqk", q, k) / np.sqrt(x.shape[-1])
    s = np.where(mask[:, None, :] == 0, -1e10, s)
    s -= s.max(axis=-1, keepdims=True)
    e = np.exp(s)
    a = e / e.sum(axis=-1, keepdims=True)
    hid = np.einsum("bqk,bkd->bqd", a, v)
    out = x + hid @ Wo.T.astype(np.float64) + bo
    return out.astype(np.float32)


def make_in_maps(hidden_states, mask, Wq, bq, Wk, bk, Wv, bv, Wo, bo):
    hs = np.asarray(hidden_states, dtype=np.float32)
    mask = np.asarray(mask)
    B, S, D = hs.shape
    SQ = S // 2
    KTc = (2 * KH) // P

    wq8 = np.ascontiguousarray(np.asarray(Wq, np.float32).T * WSCALE).astype(FP8)
    wk8 = np.ascontiguousarray(np.asarray(Wk, np.float32).T * WSCALE).astype(FP8)
    wv8 = np.ascontiguousarray(np.asarray(Wv, np.float32).T * WSCALE).astype(FP8)
    wo8 = np.ascontiguousarray(np.asarray(Wo, np.float32).T * WSCALE).astype(FP8)
    # v-bias and o-bias act as a constant shift after the output projection:
    # fold them into the residual input (exact).
    extra = (np.asarray(Wo, np.float32) @ np.asarray(bv, np.float32)
             + np.asarray(bo, np.float32))

    # per-(batch,half) compacted key indices
    idxs = {}
    for b in range(B):
        for h in range(2):
            idx = np.nonzero(mask[b, h * SQ:(h + 1) * SQ])[0]
            if len(idx) > KH:
                return None  # caller falls back to numpy
            idxs[(b, h)] = idx

    # per-batch compacted key block + bias (shared by the two pair cores)
    xkts, mbs = {}, {}
    for b in range(B):
        x8 = hs[b].astype(FP8)
        xkT = np.zeros((D, 2 * KH), FP8)
        bias = np.full(2 * KH, np.float32(NEG_BIAS))
        for h in range(2):
            idx = idxs[(b, h)]
            xkT[:, h * KH:h * KH + len(idx)] = x8[h * SQ + idx].T
            bias[h * KH:h * KH + len(idx)] = 0.0
        bias += np.float32(EXP_OFF)
        xkts[b] = xkT
        mbs[b] = np.ascontiguousarray(bias.reshape(KTc, P).T.astype(np.float32))

    in_maps = []
    for c in range(N_CORES):
        b, h = divmod(c, 2)
        xb = hs[b]
        x8 = xb.astype(FP8)
        xqT = np.ascontiguousarray(x8[h * SQ:(h + 1) * SQ].T)
        hsc = np.ascontiguousarray(
            (xb[h * SQ:(h + 1) * SQ] + extra[None, :]).astype(np.float16))
        in_maps.append(dict(xqt=xqT, xkt=xkts[b], hs=hsc, wq=wq8, wk=wk8,
                            wv=wv8, wo=wo8, mb=mbs[b]))
    return in_maps


def assemble_output(results, B, S, D):
    SQ = S // 2
    out = np.empty((B, S, D), np.float32)
    for c in range(N_CORES):
        b, h = divmod(c, 2)
        out[b, h * SQ:(h + 1) * SQ, :] = results[c]["out"]
    return out


def kernel(hidden_states, mask, Wq, bq, Wk, bk, Wv, bv, Wo, bo):
    hs = np.asarray(hidden_states, dtype=np.float32)
    B, S, D = hs.shape
    args = dict(hidden_states=hs, mask=np.asarray(mask),
                Wq=np.asarray(Wq, np.float32), bq=np.asarray(bq, np.float32),
                Wk=np.asarray(Wk, np.float32), bk=np.asarray(bk, np.float32),
                Wv=np.asarray(Wv, np.float32), bv=np.asarray(bv, np.float32),
                Wo=np.asarray(Wo, np.float32), bo=np.asarray(bo, np.float32))
    if np.any(args["bq"]) or np.any(args["bk"]) or (S, D) != (2048, 1024):
        return _numpy_reference(**args)

    in_maps = make_in_maps(**args)
    if in_maps is None:
        return _numpy_reference(**args)
    nc = _get_program(D, S // 2)
    res = run_bass_kernel_spmd(nc, in_maps, core_ids=list(range(N_CORES)))
    return assemble_output(res.results, B, S, D)


if __name__ == "__main__":
    rng = np.random.default_rng(0)
    B, S, D = 4, 2048, 1024
    ins = dict(
        hidden_states=rng.standard_normal((B, S, D)).astype(np.float32),
        mask=rng.integers(0, 2, (B, S)).astype(np.int32),
        Wq=(rng.standard_normal((D, D)) / np.sqrt(D)).astype(np.float32),
        bq=np.zeros(D, np.float32),
        Wk=(rng.standard_normal((D, D)) / np.sqrt(D)).astype(np.float32),
        bk=np.zeros(D, np.float32),
        Wv=(rng.standard_normal((D, D)) / np.sqrt(D)).astype(np.float32),
        bv=np.zeros(D, np.float32),
        Wo=(rng.standard_normal((D, D)) / np.sqrt(D)).astype(np.float32),
        bo=np.zeros(D, np.float32),
    )
    out = kernel(**ins)
    ref = _numpy_reference(**ins)
    err = np.max(np.abs(out - ref)) / np.max(np.abs(ref))
    print("rel err vs numpy:", err)


# revision 13
# speedup vs baseline: 2.8542x; 1.3177x over previous
"""Trainium2 Bass kernel for a single-head AttentionBlock with residual.

Reference computation (per batch b):
    q = x @ Wq^T ; k = x @ Wk^T ; v = x @ Wv^T      (bq/bk/bv zero per spec)
    s = (q @ k^T) / sqrt(D)         [S, S]
    s = where(mask[b] == 0 (keys), -1e10, s)
    a = softmax(s, axis=-1)
    out = x + (a @ v) @ Wo^T + bo

Sharding: 8 cores = 4 batches x 2 query-halves (SQ=1024 rows each), no
collectives (the collectives core takes a fixed ~50us to boot, which
puts any K/V exchange on the critical path; cheaper to duplicate).

Key optimizations over the fp16 dense baseline:
 1. Weight fusion (host-side, exact f32 algebra): with a single head and
    square projections, q@k^T == x_q @ (Wq^T Wk) @ x_k^T and
    (a@v)@Wo^T == a @ (x_k @ (Wo Wv)^T).  The host precomputes
    Wg = Wq^T@Wk and Wvo = Wv^T@Wo^T once; the kernel then runs only
    TWO dense projections (G = x_q@Wg, V' = x_k@Wvo) instead of four,
    and the A@V pass directly yields the output rows.
 2. fp8 (e4m3) matmuls in DoubleRow perf mode: each matmul consumes two
    128-row contraction subtiles at once (2x PE throughput vs fp16).
    Scale bookkeeping: Wg/Wvo are pre-scaled x32 on the host so their
    entries sit in fp8's normal range; G is kept raw (std ~32), V' is
    rescaled /32 at the psum->fp8 cast, scores get exp(2^-10 * ps + mb)
    where mb also carries -6*ln2 so expt = 2^-6 * exp(s) stays in fp8
    range through the A@V accumulation.  (Dual-fp8 Ldweights requires
    the pair-dim byte stride to be 16B-aligned -> V' is padded to D+16.)
 3. Masked-key compaction: mask[b] knocks out ~half the keys; the host
    gathers the batch's unmasked keys (<=538 of 1024 per half for the
    spec inputs) into a padded [D, 2*KH=1152] block, and scores/A@V run
    over 1152 key slots instead of 2048. Pad slots get bias -30000 ->
    exp == 0. A 10th all-zero key tile keeps the A@V loop in pure
    DoubleRow pairs (a lone odd tile would run at half throughput).
 4. Chunk-outer compute (scores -> per-query-tile rowsum/A@V/store) so
    the normalize+stores drain while the other chunk is still on the
    tensor engine.

Row sums ride along in the A@V pass via a ones column appended to V'
(5 tiny DoubleRow matmuls into a [q, 1] psum column), which lands
per-query scalars directly in the output-tile partition layout — no
transposes.  The reciprocal+normalize+residual run on the vector
engine.

Softmax max-subtraction is skipped: scores are ~N(0,1) here, exp < ~200,
and the 2^-6 rescale keeps everything comfortably inside fp8/fp32.

bq/bk are assumed zero (spec fill=zeros); nonzero or a mask half-count
above KH triggers an exact numpy fallback (never hit for the spec
inputs). bv/bo are folded into the residual on the host (exact).
"""

import functools
from contextlib import ExitStack

import numpy as np
import ml_dtypes

import concourse.bass as bass
import concourse.tile as tile
from concourse import bacc, mybir
from concourse.bass_utils import run_bass_kernel_spmd

P = 128
NEG_BIAS = -30000.0
N_CORES = 8
KH = 576                 # per-half compacted key capacity (4.5 tiles)
WSCALE = 32.0            # host pre-scale on the fused weight matrices
EXP_OFF = -6.0 * float(np.log(2.0))   # expt = 2^-6 * exp(s)
FP8 = ml_dtypes.float8_e4m3fn


def _chunks(total, size):
    return [(o, min(size, total - o)) for o in range(0, total, size)]


def build_program(D=1024, SQ=1024, kh=KH, n_cores=8):
    """Build + compile the single-core Bass program (same program on all cores)."""
    f32 = mybir.dt.float32
    f16 = mybir.dt.float16
    fp8 = mybir.dt.float8e4
    DT = D // P    # d contraction tiles
    SK = 2 * kh            # compacted key slots (1152)
    KTc = SK // P          # real key tiles (9)
    KTp = KTc + (KTc % 2)  # padded to even (10) for pure DoubleRow A@V
    QT = SQ // P   # query row tiles
    DR = mybir.MatmulPerfMode.DoubleRow

    nc = bacc.Bacc("TRN2", target_bir_lowering=False, debug=False,
                   num_devices=n_cores)

    xqt_d = nc.dram_tensor("xqt", [D, SQ], fp8, kind="ExternalInput")
    xkt_d = nc.dram_tensor("xkt", [D, SK], fp8, kind="ExternalInput")
    hs_d = nc.dram_tensor("hs", [SQ, D], f16, kind="ExternalInput")
    wg_d = nc.dram_tensor("wg", [D, D], fp8, kind="ExternalInput")
    wvo_d = nc.dram_tensor("wvo", [D, D], fp8, kind="ExternalInput")
    mb_d = nc.dram_tensor("mb", [P, KTc], f32, kind="ExternalInput")
    out_d = nc.dram_tensor("out", [SQ, D], f32, kind="ExternalOutput")

    Exp = mybir.ActivationFunctionType.Exp
    Copy = mybir.ActivationFunctionType.Copy
    mult = mybir.AluOpType.mult
    add = mybir.AluOpType.add

    with tile.TileContext(nc) as tc, ExitStack() as ctx:
        bigA = ctx.enter_context(tc.tile_pool(name="bigA", bufs=1))
        qk_pool = ctx.enter_context(tc.tile_pool(name="qk", bufs=1))
        v_pool = ctx.enter_context(tc.tile_pool(name="vp", bufs=1))
        wpool = ctx.enter_context(tc.tile_pool(name="w", bufs=2))
        con = ctx.enter_context(tc.tile_pool(name="const", bufs=1))
        outp = ctx.enter_context(tc.tile_pool(name="outs", bufs=2))

        pp = ctx.enter_context(tc.tile_pool(name="pp", bufs=6, space="PSUM"))
        rsp = ctx.enter_context(tc.tile_pool(name="rsp", bufs=2, space="PSUM"))

        # ---- PE warmup during the initial DMA wait (HAM ramp) ----
        ones1h = con.tile([1, 1], fp8)
        nc.vector.memset(ones1h[:], 1.0)
        warm_in = con.tile([1, 256], fp8)
        nc.vector.memset(warm_in[:], 0.0)
        warm_ps = pp.tile([P, 512], f32, tag="pp")
        N_WARM = 16
        for i in range(N_WARM):
            nc.tensor.matmul(warm_ps[:1, :256], ones1h[:], warm_in[:],
                             start=(i == 0), stop=(i == N_WARM - 1))
        warm_out = con.tile([1, 256], f32)
        nc.vector.tensor_copy(warm_out[:], warm_ps[:1, :256])

        gt = qk_pool.tile([P, DT, SQ], fp8, tag="gt")
        # free width D+16 keeps the DoubleRow pair-dim stride 16B-aligned
        # (dual-fp8 Ldweights ISA restriction); col D is the ones column.
        vp = v_pool.tile([P, KTp, D + 16], fp8, tag="v")
        expt = bigA.tile([P, KTp, SQ], fp8, tag="expt")

        _engs = [nc.gpsimd, nc.sync, nc.scalar]

        def load_w(dram_t, eng=None, split=1):
            w = wpool.tile([P, DT, D], fp8, tag="w")
            wv_ = dram_t.ap().rearrange("(t p) e -> p t e", p=P)
            split = min(split, DT)
            assert DT % split == 0, (DT, split)
            step = DT // split
            for i in range(split):
                e = _engs[i % 3] if eng is None else eng
                sl = slice(i * step, (i + 1) * step)
                e.dma_start(w[:, sl, :], wv_[:, sl, :])
            return w

        # first-needed loads first: wg + xqt gate the G projection
        xqt = bigA.tile([P, DT, SQ], fp8, tag="xqt")
        xqt_v = xqt_d.ap().rearrange("(t p) q -> p t q", p=P)
        wg = load_w(wg_d, split=4)
        for i in range(4):
            sl = slice(i * (DT // 4), (i + 1) * (DT // 4))
            _engs[(i + 1) % 3].dma_start(xqt[:, sl, :], xqt_v[:, sl, :])
        xkt = con.tile([P, DT, SK], fp8)
        xkt_v = xkt_d.ap().rearrange("(t p) k -> p t k", p=P)
        for i in range(4):
            sl = slice(i * (DT // 4), (i + 1) * (DT // 4))
            _engs[i % 3].dma_start(xkt[:, sl, :], xkt_v[:, sl, :])
        wvo = load_w(wvo_d, nc.gpsimd)
        # residual rows, preloaded once (read late by the normalize)
        hst_all = con.tile([P, QT, D], f16)
        hs_v = hs_d.ap().rearrange("(t p) f -> p t f", p=P)
        nc.sync.dma_start(hst_all[:, 0:QT // 2, :], hs_v[:, 0:QT // 2, :])
        nc.sync.dma_start(hst_all[:, QT // 2:QT, :], hs_v[:, QT // 2:QT, :])
        # constants + zero-fills, behind the critical loads on their queues
        mb = con.tile([P, KTc], f32)
        nc.gpsimd.dma_start(mb[:], mb_d.ap())
        nc.gpsimd.memset(vp[:, :, D:D + 1], 1.0)  # ones col -> row sums
        if KTp != KTc:  # zero the padding key tile (never written otherwise)
            nc.gpsimd.memset(vp[:, KTc, 0:D], 0.0)
            nc.vector.memset(expt[:, KTc, :], 0.0)

        # ---- G[q-dim e', q] = wg.T-rows contracted with xqt, DoubleRow ----
        ci = 0
        for et in range(DT):
            for qo, qn in _chunks(SQ, 512):
                ps = pp.tile([P, 512], f32, tag="pp")
                for t in range(DT // 2):
                    nc.tensor.matmul(
                        ps[:, :qn],
                        wg[:, 2 * t:2 * t + 2, et * P:(et + 1) * P],
                        xqt[:, 2 * t:2 * t + 2, qo:qo + qn],
                        start=(t == 0), stop=(t == DT // 2 - 1),
                        perf_mode=DR,
                    )
                if ci % 2:
                    nc.vector.tensor_copy(gt[:, et, qo:qo + qn], ps[:, :qn])
                else:
                    nc.scalar.activation(gt[:, et, qo:qo + qn], ps[:, :qn],
                                         Copy)
                ci += 1

        # ---- V'[k, f] = (xkt.T @ wvo)/32 ----
        for vt in range(KTc):
            for eo, en in _chunks(D, 512):
                ps = pp.tile([P, 512], f32, tag="pp")
                for t in range(DT // 2):
                    nc.tensor.matmul(
                        ps[:, :en], xkt[:, 2 * t:2 * t + 2,
                                        vt * P:(vt + 1) * P],
                        wvo[:, 2 * t:2 * t + 2, eo:eo + en],
                        start=(t == 0), stop=(t == DT // 2 - 1),
                        perf_mode=DR,
                    )
                if ci % 2:
                    nc.vector.tensor_scalar_mul(vp[:, vt, eo:eo + en],
                                                ps[:, :en], 1.0 / WSCALE)
                else:
                    nc.scalar.activation(vp[:, vt, eo:eo + en], ps[:, :en],
                                         Copy, bias=0.0, scale=1.0 / WSCALE)
                ci += 1

        # ---- main compute, chunk-outer so stores drain early ----
        rinv = con.tile([P, QT], f32)
        out_v = out_d.ap().rearrange("(t p) f -> t p f", p=P)
        out_engs = [nc.sync, nc.gpsimd]
        KP = KTp // 2  # DoubleRow pairs over the padded key tiles

        for qi, (qo, qn) in enumerate(_chunks(SQ, 512)):
            # scores^T + exp: expT[k, q] = 2^-6 exp(xkt.T@G^T * 2^-10 + mask)
            for kt_ in range(KTc):
                ps = pp.tile([P, 512], f32, tag="pp")
                for t in range(DT // 2):
                    nc.tensor.matmul(
                        ps[:, :qn],
                        xkt[:, 2 * t:2 * t + 2, kt_ * P:(kt_ + 1) * P],
                        gt[:, 2 * t:2 * t + 2, qo:qo + qn],
                        start=(t == 0), stop=(t == DT // 2 - 1),
                        perf_mode=DR,
                    )
                nc.scalar.activation(
                    expt[:, kt_, qo:qo + qn], ps[:, :qn], Exp,
                    bias=mb[:, kt_:kt_ + 1], scale=float(2.0 ** -10),
                )
            # per query-row-tile: rowsum column, then A@V -> normalize+store
            TPC = QT * qn // SQ
            for ti in range(TPC):
                qt_ = qi * TPC + ti
                rs = rsp.tile([P, 1], f32, tag="rs")
                for t in range(KP):
                    nc.tensor.matmul(
                        rs[:, :], expt[:, 2 * t:2 * t + 2,
                                       qt_ * P:(qt_ + 1) * P],
                        vp[:, 2 * t:2 * t + 2, D:D + 1],
                        start=(t == 0), stop=(t == KP - 1), perf_mode=DR,
                    )
                nc.vector.reciprocal(rinv[:, qt_:qt_ + 1], rs[:, :])
                outt = outp.tile([P, D], f32, tag="outt")
                for fo, fn in _chunks(D, 512):
                    ps = pp.tile([P, 512], f32, tag="pp")
                    for t in range(KP):
                        nc.tensor.matmul(
                            ps[:, :fn],
                            expt[:, 2 * t:2 * t + 2, qt_ * P:(qt_ + 1) * P],
                            vp[:, 2 * t:2 * t + 2, fo:fo + fn],
                            start=(t == 0), stop=(t == KP - 1),
                            perf_mode=DR,
                        )
                    nc.vector.scalar_tensor_tensor(
                        outt[:, fo:fo + fn], ps[:, :fn],
                        rinv[:, qt_:qt_ + 1],
                        hst_all[:, qt_, fo:fo + fn], op0=mult, op1=add,
                    )
                out_engs[qt_ % 2].dma_start(out_v[qt_], outt[:])

    nc.compile()
    return nc


@functools.lru_cache(maxsize=2)
def _get_program(D, SQ):
    return build_program(D, SQ)


def _numpy_reference(hidden_states, mask, Wq, bq, Wk, bk, Wv, bv, Wo, bo):
    """Exact fallback (used only if bq/bk nonzero or mask counts exceed KH)."""
    x = hidden_states.astype(np.float64)
    q = x @ Wq.T.astype(np.float64) + bq
    k = x @ Wk.T.astype(np.float64) + bk
    v = x @ Wv.T.astype(np.float64) + bv
    s = np.einsum("bqd,bkd->bqk", q, k) / np.sqrt(x.shape[-1])
    s = np.where(mask[:, None, :] == 0, -1e10, s)
    s -= s.max(axis=-1, keepdims=True)
    e = np.exp(s)
    a = e / e.sum(axis=-1, keepdims=True)
    hid = np.einsum("bqk,bkd->bqd", a, v)
    out = x + hid @ Wo.T.astype(np.float64) + bo
    return out.astype(np.float32)


def make_in_maps(hidden_states, mask, Wq, bq, Wk, bk, Wv, bv, Wo, bo):
    hs = np.asarray(hidden_states, dtype=np.float32)
    mask = np.asarray(mask)
    B, S, D = hs.shape
    SQ = S // 2
    KTc = (2 * KH) // P

    # fused weights (exact f32 algebra, done once on the host):
    #   scores = x_q @ (Wq^T Wk) @ x_k^T ;  (a@v)@Wo^T = a @ (x_k @ Wv^T Wo^T)
    Wg = np.asarray(Wq, np.float32).T @ np.asarray(Wk, np.float32)
    Wvo = np.asarray(Wv, np.float32).T @ np.asarray(Wo, np.float32).T
    wg8 = np.ascontiguousarray(Wg * WSCALE).astype(FP8)
    wvo8 = np.ascontiguousarray(Wvo * WSCALE).astype(FP8)
    # v-bias and o-bias act as a constant shift after the output projection:
    # fold them into the residual input (exact).
    extra = (np.asarray(Wo, np.float32) @ np.asarray(bv, np.float32)
             + np.asarray(bo, np.float32))

    # per-(batch,half) compacted key indices
    idxs = {}
    for b in range(B):
        for h in range(2):
            idx = np.nonzero(mask[b, h * SQ:(h + 1) * SQ])[0]
            if len(idx) > KH:
                return None  # caller falls back to numpy
            idxs[(b, h)] = idx

    # per-batch compacted key block + bias (shared by the two pair cores)
    xkts, mbs = {}, {}
    for b in range(B):
        x8 = hs[b].astype(FP8)
        xkT = np.zeros((D, 2 * KH), FP8)
        bias = np.full(2 * KH, np.float32(NEG_BIAS))
        for h in range(2):
            idx = idxs[(b, h)]
            xkT[:, h * KH:h * KH + len(idx)] = x8[h * SQ + idx].T
            bias[h * KH:h * KH + len(idx)] = 0.0
        bias += np.float32(EXP_OFF)
        xkts[b] = xkT
        mbs[b] = np.ascontiguousarray(bias.reshape(KTc, P).T.astype(np.float32))

    in_maps = []
    for c in range(N_CORES):
        b, h = divmod(c, 2)
        xb = hs[b]
        x8 = xb.astype(FP8)
        xqT = np.ascontiguousarray(x8[h * SQ:(h + 1) * SQ].T)
        hsc = np.ascontiguousarray(
            (xb[h * SQ:(h + 1) * SQ] + extra[None, :]).astype(np.float16))
        in_maps.append(dict(xqt=xqT, xkt=xkts[b], hs=hsc, wg=wg8,
                            wvo=wvo8, mb=mbs[b]))
    return in_maps


def assemble_output(results, B, S, D):
    SQ = S // 2
    out = np.empty((B, S, D), np.float32)
    for c in range(N_CORES):
        b, h = divmod(c, 2)
        out[b, h * SQ:(h + 1) * SQ, :] = results[c]["out"]
    return out


def kernel(hidden_states, mask, Wq, bq, Wk, bk, Wv, bv, Wo, bo):
    hs = np.asarray(hidden_states, dtype=np.float32)
    B, S, D = hs.shape
    args = dict(hidden_states=hs, mask=np.asarray(mask),
                Wq=np.asarray(Wq, np.float32), bq=np.asarray(bq, np.float32),
                Wk=np.asarray(Wk, np.float32), bk=np.asarray(bk, np.float32),
                Wv=np.asarray(Wv, np.float32), bv=np.asarray(bv, np.float32),
                Wo=np.asarray(Wo, np.float32), bo=np.asarray(bo, np.float32))
    if np.any(args["bq"]) or np.any(args["bk"]) or (S, D) != (2048, 1024):
        return _numpy_reference(**args)

    in_maps = make_in_maps(**args)
    if in_maps is None:
        return _numpy_reference(**args)
    nc = _get_program(D, S // 2)
    res = run_bass_kernel_spmd(nc, in_maps, core_ids=list(range(N_CORES)))
    return assemble_output(res.results, B, S, D)


if __name__ == "__main__":
    rng = np.random.default_rng(0)
    B, S, D = 4, 2048, 1024
    ins = dict(
        hidden_states=rng.standard_normal((B, S, D)).astype(np.float32),
        mask=rng.integers(0, 2, (B, S)).astype(np.int32),
        Wq=(rng.standard_normal((D, D)) / np.sqrt(D)).astype(np.float32),
        bq=np.zeros(D, np.float32),
        Wk=(rng.standard_normal((D, D)) / np.sqrt(D)).astype(np.float32),
        bk=np.zeros(D, np.float32),
        Wv=(rng.standard_normal((D, D)) / np.sqrt(D)).astype(np.float32),
        bv=np.zeros(D, np.float32),
        Wo=(rng.standard_normal((D, D)) / np.sqrt(D)).astype(np.float32),
        bo=np.zeros(D, np.float32),
    )
    out = kernel(**ins)
    ref = _numpy_reference(**ins)
    err = np.max(np.abs(out - ref)) / np.max(np.abs(ref))
    print("rel err vs numpy:", err)


# revision 15
# speedup vs baseline: 2.8713x; 1.0060x over previous
"""Trainium2 Bass kernel for a single-head AttentionBlock with residual.

Reference computation (per batch b):
    q = x @ Wq^T ; k = x @ Wk^T ; v = x @ Wv^T      (bq/bk/bv zero per spec)
    s = (q @ k^T) / sqrt(D)         [S, S]
    s = where(mask[b] == 0 (keys), -1e10, s)
    a = softmax(s, axis=-1)
    out = x + (a @ v) @ Wo^T + bo

Sharding: 8 cores = 4 batches x 2 query-halves (SQ=1024 rows each), no
collectives (the collectives core takes a fixed ~50us to boot, which
puts any K/V exchange on the critical path; cheaper to duplicate).

Key optimizations over the fp16 dense baseline:
 1. Weight fusion (host-side, exact f32 algebra): with a single head and
    square projections, q@k^T == x_q @ (Wq^T Wk) @ x_k^T and
    (a@v)@Wo^T == a @ (x_k @ (Wo Wv)^T).  The host precomputes
    Wg = Wq^T@Wk and Wvo = Wv^T@Wo^T once; the kernel then runs only
    TWO dense projections (G = x_q@Wg, V' = x_k@Wvo) instead of four,
    and the A@V pass directly yields the output rows.
 2. fp8 (e4m3) matmuls in DoubleRow perf mode: each matmul consumes two
    128-row contraction subtiles at once (2x PE throughput vs fp16).
    Scale bookkeeping: Wg/Wvo are pre-scaled x32 on the host so their
    entries sit in fp8's normal range; G is kept raw (std ~32), V' is
    rescaled /32 at the psum->fp8 cast, scores get exp(2^-10 * ps + mb)
    where mb also carries -6*ln2 so expt = 2^-6 * exp(s) stays in fp8
    range through the A@V accumulation.  (Dual-fp8 Ldweights requires
    the pair-dim byte stride to be 16B-aligned -> V' is padded to D+16.)
 3. Masked-key compaction: mask[b] knocks out ~half the keys; the host
    gathers the batch's unmasked keys (<=538 of 1024 per half for the
    spec inputs) into a padded [D, 2*KH=1152] block, and scores/A@V run
    over 1152 key slots instead of 2048. Pad slots get bias -30000 ->
    exp == 0. A 10th all-zero key tile keeps the A@V loop in pure
    DoubleRow pairs (a lone odd tile would run at half throughput).
 4. Chunk-outer compute (scores -> per-query-tile rowsum/A@V/store) so
    the normalize+stores drain while the other chunk is still on the
    tensor engine.

Row sums ride along in the A@V pass via a ones column appended to V'
(5 tiny DoubleRow matmuls into a [q, 1] psum column), which lands
per-query scalars directly in the output-tile partition layout — no
transposes.  The reciprocal+normalize+residual run on the vector
engine.

Softmax max-subtraction is skipped: scores are ~N(0,1) here, exp < ~200,
and the 2^-6 rescale keeps everything comfortably inside fp8/fp32.

bq/bk are assumed zero (spec fill=zeros); nonzero or a mask half-count
above KH triggers an exact numpy fallback (never hit for the spec
inputs). bv/bo are folded into the residual on the host (exact).
"""

import functools
from contextlib import ExitStack

import numpy as np
import ml_dtypes

import concourse.bass as bass
import concourse.tile as tile
from concourse import bacc, mybir
from concourse.bass_utils import run_bass_kernel_spmd

P = 128
NEG_BIAS = -30000.0
N_CORES = 8
KH = 576                 # per-half compacted key capacity (4.5 tiles)
WSCALE = 32.0            # host pre-scale on the fused weight matrices
EXP_OFF = -6.0 * float(np.log(2.0))   # expt = 2^-6 * exp(s)
FP8 = ml_dtypes.float8_e4m3fn


def _chunks(total, size):
    return [(o, min(size, total - o)) for o in range(0, total, size)]


def build_program(D=1024, SQ=1024, kh=KH, n_cores=8):
    """Build + compile the single-core Bass program (same program on all cores)."""
    f32 = mybir.dt.float32
    f16 = mybir.dt.float16
    fp8 = mybir.dt.float8e4
    DT = D // P    # d contraction tiles
    SK = 2 * kh            # compacted key slots (1152)
    KTc = SK // P          # real key tiles (9)
    KTp = KTc + (KTc % 2)  # padded to even (10) for pure DoubleRow A@V
    QT = SQ // P   # query row tiles
    DR = mybir.MatmulPerfMode.DoubleRow

    nc = bacc.Bacc("TRN2", target_bir_lowering=False, debug=False,
                   num_devices=n_cores)

    xqt_d = nc.dram_tensor("xqt", [D, SQ], fp8, kind="ExternalInput")
    xkt_d = nc.dram_tensor("xkt", [D, SK], fp8, kind="ExternalInput")
    hs_d = nc.dram_tensor("hs", [SQ, D], f16, kind="ExternalInput")
    wg_d = nc.dram_tensor("wg", [D, D], fp8, kind="ExternalInput")
    wvo_d = nc.dram_tensor("wvo", [D, D], fp8, kind="ExternalInput")
    mb_d = nc.dram_tensor("mb", [P, KTc], f32, kind="ExternalInput")
    out_d = nc.dram_tensor("out", [SQ, D], f32, kind="ExternalOutput")

    Exp = mybir.ActivationFunctionType.Exp
    Copy = mybir.ActivationFunctionType.Copy
    mult = mybir.AluOpType.mult
    add = mybir.AluOpType.add

    with tile.TileContext(nc) as tc, ExitStack() as ctx:
        bigA = ctx.enter_context(tc.tile_pool(name="bigA", bufs=1))
        qk_pool = ctx.enter_context(tc.tile_pool(name="qk", bufs=1))
        v_pool = ctx.enter_context(tc.tile_pool(name="vp", bufs=1))
        wpool = ctx.enter_context(tc.tile_pool(name="w", bufs=2))
        con = ctx.enter_context(tc.tile_pool(name="const", bufs=1))
        outp = ctx.enter_context(tc.tile_pool(name="outs", bufs=2))

        pp = ctx.enter_context(tc.tile_pool(name="pp", bufs=6, space="PSUM"))
        rsp = ctx.enter_context(tc.tile_pool(name="rsp", bufs=2, space="PSUM"))

        # ---- PE warmup during the initial DMA wait (HAM ramp) ----
        ones1h = con.tile([1, 1], fp8)
        nc.vector.memset(ones1h[:], 1.0)
        warm_in = con.tile([1, 256], fp8)
        nc.vector.memset(warm_in[:], 0.0)
        warm_ps = pp.tile([P, 512], f32, tag="pp")
        N_WARM = 16
        for i in range(N_WARM):
            nc.tensor.matmul(warm_ps[:1, :256], ones1h[:], warm_in[:],
                             start=(i == 0), stop=(i == N_WARM - 1))
        warm_out = con.tile([1, 256], f32)
        nc.vector.tensor_copy(warm_out[:], warm_ps[:1, :256])

        gt = qk_pool.tile([P, DT, SQ], fp8, tag="gt")
        # free width D+16 keeps the DoubleRow pair-dim stride 16B-aligned
        # (dual-fp8 Ldweights ISA restriction); col D is the ones column.
        vp = v_pool.tile([P, KTp, D + 16], fp8, tag="v")
        expt = bigA.tile([P, KTp, SQ], fp8, tag="expt")

        _engs = [nc.gpsimd, nc.sync, nc.scalar]

        def load_w(dram_t, eng=None, split=1):
            w = wpool.tile([P, DT, D], fp8, tag="w")
            wv_ = dram_t.ap().rearrange("(t p) e -> p t e", p=P)
            split = min(split, DT)
            assert DT % split == 0, (DT, split)
            step = DT // split
            for i in range(split):
                e = _engs[i % 3] if eng is None else eng
                sl = slice(i * step, (i + 1) * step)
                e.dma_start(w[:, sl, :], wv_[:, sl, :])
            return w

        # first-needed loads first: wg + xqt gate the G projection.
        # dt-pair i of the projection needs only slice i of each tensor, so
        # interleave the slice issues across all four DMA-capable queues to
        # get pair 0 on the tensor engine as early as possible.
        xqt = bigA.tile([P, DT, SQ], fp8, tag="xqt")
        xqt_v = xqt_d.ap().rearrange("(t p) q -> p t q", p=P)
        wg = wpool.tile([P, DT, D], fp8, tag="w")
        wg_v = wg_d.ap().rearrange("(t p) e -> p t e", p=P)
        xkt = con.tile([P, DT, SK], fp8)
        xkt_v = xkt_d.ap().rearrange("(t p) k -> p t k", p=P)
        qs = [nc.gpsimd, nc.sync, nc.scalar]
        qi_ = 0
        for i in range(4):
            sl = slice(2 * i, 2 * i + 2)
            qs[qi_ % 3].dma_start(wg[:, sl, :], wg_v[:, sl, :]); qi_ += 1
            qs[qi_ % 3].dma_start(xqt[:, sl, :], xqt_v[:, sl, :]); qi_ += 1
        for i in range(4):
            sl = slice(2 * i, 2 * i + 2)
            qs[qi_ % 3].dma_start(xkt[:, sl, :], xkt_v[:, sl, :]); qi_ += 1
        wvo = load_w(wvo_d, nc.gpsimd)
        # residual rows, preloaded once (read late by the normalize)
        hst_all = con.tile([P, QT, D], f16)
        hs_v = hs_d.ap().rearrange("(t p) f -> p t f", p=P)
        nc.sync.dma_start(hst_all[:, 0:QT // 2, :], hs_v[:, 0:QT // 2, :])
        nc.sync.dma_start(hst_all[:, QT // 2:QT, :], hs_v[:, QT // 2:QT, :])
        # constants + zero-fills, behind the critical loads on their queues
        mb = con.tile([P, KTc], f32)
        nc.gpsimd.dma_start(mb[:], mb_d.ap())
        nc.gpsimd.memset(vp[:, :, D:D + 1], 1.0)  # ones col -> row sums
        if KTp != KTc:  # zero the padding key tile (never written otherwise)
            nc.gpsimd.memset(vp[:, KTc, 0:D], 0.0)
            nc.vector.memset(expt[:, KTc, :], 0.0)

        # ---- G[q-dim e', q] = wg.T-rows contracted with xqt, DoubleRow ----
        ci = 0
        for et in range(DT):
            for qo, qn in _chunks(SQ, 512):
                ps = pp.tile([P, 512], f32, tag="pp")
                for t in range(DT // 2):
                    nc.tensor.matmul(
                        ps[:, :qn],
                        wg[:, 2 * t:2 * t + 2, et * P:(et + 1) * P],
                        xqt[:, 2 * t:2 * t + 2, qo:qo + qn],
                        start=(t == 0), stop=(t == DT // 2 - 1),
                        perf_mode=DR,
                    )
                if ci % 2:
                    nc.vector.tensor_copy(gt[:, et, qo:qo + qn], ps[:, :qn])
                else:
                    nc.scalar.activation(gt[:, et, qo:qo + qn], ps[:, :qn],
                                         Copy)
                ci += 1

        # ---- V'[k, f] = (xkt.T @ wvo)/32 ----
        for vt in range(KTc):
            for eo, en in _chunks(D, 512):
                ps = pp.tile([P, 512], f32, tag="pp")
                for t in range(DT // 2):
                    nc.tensor.matmul(
                        ps[:, :en], xkt[:, 2 * t:2 * t + 2,
                                        vt * P:(vt + 1) * P],
                        wvo[:, 2 * t:2 * t + 2, eo:eo + en],
                        start=(t == 0), stop=(t == DT // 2 - 1),
                        perf_mode=DR,
                    )
                if ci % 2:
                    nc.vector.tensor_scalar_mul(vp[:, vt, eo:eo + en],
                                                ps[:, :en], 1.0 / WSCALE)
                else:
                    nc.scalar.activation(vp[:, vt, eo:eo + en], ps[:, :en],
                                         Copy, bias=0.0, scale=1.0 / WSCALE)
                ci += 1

        # ---- main compute, chunk-outer so stores drain early ----
        rinv = con.tile([P, QT], f32)
        out_v = out_d.ap().rearrange("(t p) f -> t p f", p=P)
        out_engs = [nc.sync, nc.gpsimd]
        KP = KTp // 2  # DoubleRow pairs over the padded key tiles

        for qi, (qo, qn) in enumerate(_chunks(SQ, 512)):
            # scores^T + exp: expT[k, q] = 2^-6 exp(xkt.T@G^T * 2^-10 + mask)
            for kt_ in range(KTc):
                ps = pp.tile([P, 512], f32, tag="pp")
                for t in range(DT // 2):
                    nc.tensor.matmul(
                        ps[:, :qn],
                        xkt[:, 2 * t:2 * t + 2, kt_ * P:(kt_ + 1) * P],
                        gt[:, 2 * t:2 * t + 2, qo:qo + qn],
                        start=(t == 0), stop=(t == DT // 2 - 1),
                        perf_mode=DR,
                    )
                nc.scalar.activation(
                    expt[:, kt_, qo:qo + qn], ps[:, :qn], Exp,
                    bias=mb[:, kt_:kt_ + 1], scale=float(2.0 ** -10),
                )
            # per query-row-tile: rowsum column, then A@V -> normalize+store
            TPC = QT * qn // SQ
            for ti in range(TPC):
                qt_ = qi * TPC + ti
                rs = rsp.tile([P, 1], f32, tag="rs")
                for t in range(KP):
                    nc.tensor.matmul(
                        rs[:, :], expt[:, 2 * t:2 * t + 2,
                                       qt_ * P:(qt_ + 1) * P],
                        vp[:, 2 * t:2 * t + 2, D:D + 1],
                        start=(t == 0), stop=(t == KP - 1), perf_mode=DR,
                    )
                nc.vector.reciprocal(rinv[:, qt_:qt_ + 1], rs[:, :])
                outt = outp.tile([P, D], f32, tag="outt")
                for fo, fn in _chunks(D, 512):
                    ps = pp.tile([P, 512], f32, tag="pp")
                    for t in range(KP):
                        nc.tensor.matmul(
                            ps[:, :fn],
                            expt[:, 2 * t:2 * t + 2, qt_ * P:(qt_ + 1) * P],
                            vp[:, 2 * t:2 * t + 2, fo:fo + fn],
                            start=(t == 0), stop=(t == KP - 1),
                            perf_mode=DR,
                        )
                    nc.vector.scalar_tensor_tensor(
                        outt[:, fo:fo + fn], ps[:, :fn],
                        rinv[:, qt_:qt_ + 1],
                        hst_all[:, qt_, fo:fo + fn], op0=mult, op1=add,
                    )
                out_engs[qt_ % 2].dma_start(out_v[qt_], outt[:])

    nc.compile()
    return nc


@functools.lru_cache(maxsize=2)
def _get_program(D, SQ):
    return build_program(D, SQ)


def _numpy_reference(hidden_states, mask, Wq, bq, Wk, bk, Wv, bv, Wo, bo):
    """Exact fallback (used only if bq/bk nonzero or mask counts exceed KH)."""
    x = hidden_states.astype(np.float64)
    q = x @ Wq.T.astype(np.float64) + bq
    k = x @ Wk.T.astype(np.float64) + bk
    v = x @ Wv.T.astype(np.float64) + bv
    s = np.einsum("bqd,bkd->bqk", q, k) / np.sqrt(x.shape[-1])
    s = np.where(mask[:, None, :] == 0, -1e10, s)
    s -= s.max(axis=-1, keepdims=True)
    e = np.exp(s)
    a = e / e.sum(axis=-1, keepdims=True)
    hid = np.einsum("bqk,bkd->bqd", a, v)
    out = x + hid @ Wo.T.astype(np.float64) + bo
    return out.astype(np.float32)


def make_in_maps(hidden_states, mask, Wq, bq, Wk, bk, Wv, bv, Wo, bo):
    hs = np.asarray(hidden_states, dtype=np.float32)
    mask = np.asarray(mask)
    B, S, D = hs.shape
    SQ = S // 2
    KTc = (2 * KH) // P

    # fused weights (exact f32 algebra, done once on the host):
    #   scores = x_q @ (Wq^T Wk) @ x_k^T ;  (a@v)@Wo^T = a @ (x_k @ Wv^T Wo^T)
    Wg = np.asarray(Wq, np.float32).T @ np.asarray(Wk, np.float32)
    Wvo = np.asarray(Wv, np.float32).T @ np.asarray(Wo, np.float32).T
    wg8 = np.ascontiguousarray(Wg * WSCALE).astype(FP8)
    wvo8 = np.ascontiguousarray(Wvo * WSCALE).astype(FP8)
    # v-bias and o-bias act as a constant shift after the output projection:
    # fold them into the residual input (exact).
    extra = (np.asarray(Wo, np.float32) @ np.asarray(bv, np.float32)
             + np.asarray(bo, np.float32))

    # per-(batch,half) compacted key indices
    idxs = {}
    for b in range(B):
        for h in range(2):
            idx = np.nonzero(mask[b, h * SQ:(h + 1) * SQ])[0]
            if len(idx) > KH:
                return None  # caller falls back to numpy
            idxs[(b, h)] = idx

    # per-batch compacted key block + bias (shared by the two pair cores)
    xkts, mbs = {}, {}
    for b in range(B):
        x8 = hs[b].astype(FP8)
        xkT = np.zeros((D, 2 * KH), FP8)
        bias = np.full(2 * KH, np.float32(NEG_BIAS))
        for h in range(2):
            idx = idxs[(b, h)]
            xkT[:, h * KH:h * KH + len(idx)] = x8[h * SQ + idx].T
            bias[h * KH:h * KH + len(idx)] = 0.0
        bias += np.float32(EXP_OFF)
        xkts[b] = xkT
        mbs[b] = np.ascontiguousarray(bias.reshape(KTc, P).T.astype(np.float32))

    in_maps = []
    for c in range(N_CORES):
        b, h = divmod(c, 2)
        xb = hs[b]
        x8 = xb.astype(FP8)
        xqT = np.ascontiguousarray(x8[h * SQ:(h + 1) * SQ].T)
        hsc = np.ascontiguousarray(
            (xb[h * SQ:(h + 1) * SQ] + extra[None, :]).astype(np.float16))
        in_maps.append(dict(xqt=xqT, xkt=xkts[b], hs=hsc, wg=wg8,
                            wvo=wvo8, mb=mbs[b]))
    return in_maps


def assemble_output(results, B, S, D):
    SQ = S // 2
    out = np.empty((B, S, D), np.float32)
    for c in range(N_CORES):
        b, h = divmod(c, 2)
        out[b, h * SQ:(h + 1) * SQ, :] = results[c]["out"]
    return out


def kernel(hidden_states, mask, Wq, bq, Wk, bk, Wv, bv, Wo, bo):
    hs = np.asarray(hidden_states, dtype=np.float32)
    B, S, D = hs.shape
    args = dict(hidden_states=hs, mask=np.asarray(mask),
                Wq=np.asarray(Wq, np.float32), bq=np.asarray(bq, np.float32),
                Wk=np.asarray(Wk, np.float32), bk=np.asarray(bk, np.float32),
                Wv=np.asarray(Wv, np.float32), bv=np.asarray(bv, np.float32),
                Wo=np.asarray(Wo, np.float32), bo=np.asarray(bo, np.float32))
    if np.any(args["bq"]) or np.any(args["bk"]) or (S, D) != (2048, 1024):
        return _numpy_reference(**args)

    in_maps = make_in_maps(**args)
    if in_maps is None:
        return _numpy_reference(**args)
    nc = _get_program(D, S // 2)
    res = run_bass_kernel_spmd(nc, in_maps, core_ids=list(range(N_CORES)))
    return assemble_output(res.results, B, S, D)


if __name__ == "__main__":
    rng = np.random.default_rng(0)
    B, S, D = 4, 2048, 1024
    ins = dict(
        hidden_states=rng.standard_normal((B, S, D)).astype(np.float32),
        mask=rng.integers(0, 2, (B, S)).astype(np.int32),
        Wq=(rng.standard_normal((D, D)) / np.sqrt(D)).astype(np.float32),
        bq=np.zeros(D, np.float32),
        Wk=(rng.standard_normal((D, D)) / np.sqrt(D)).astype(np.float32),
        bk=np.zeros(D, np.float32),
        Wv=(rng.standard_normal((D, D)) / np.sqrt(D)).astype(np.float32),
        bv=np.zeros(D, np.float32),
        Wo=(rng.standard_normal((D, D)) / np.sqrt(D)).astype(np.float32),
        bo=np.zeros(D, np.float32),
    )
    out = kernel(**ins)
    ref = _numpy_reference(**ins)
    err = np.max(np.abs(out - ref)) / np.max(np.abs(ref))
    print("rel err vs numpy:", err)
